# revision 21
# baseline (speedup 1.0000x reference)
"""Trainium2 Bass kernel for nn_Decoder_20486994002617.  v3.

8-core tensor-parallel 2-layer llama-style decoder with ragged token-merge
(handled on host), returning the masked-mean cross-entropy loss.

v2: fp8e4 DoubleRow for qkv / o / gate / up / down / lm_head, weights
pre-scaled (x64, up-proj x4) into e4m3 range, compensation folded into rope
constants and scaled PSUM->SBUF copies; host pre-chunks weights into SBUF
tile layout so streaming DMAs are contiguous.

v3 (latency restructure):
  - AllReduces quartered (256 rows each) and posted as soon as their rows
    are ready, so consumers never wait on a half-sequence collective.
  - A persistent "bridge" array xnt[0..7] holds the normalized transposed
    activations; the residual-add + rmsnorm + transpose for each phase is
    emitted inside the *previous* phase's instruction stream (prep), so
    GEMMs start immediately at phase entry.
  - lm head uses a fixed-max (M=16) online softmax: no logits storage, no
    max AllReduce, no DRAM spill of h.
  - tlog runs in sequence halves so it can start before the last xf tiles.
"""
import numpy as np
import ml_dtypes

from contextlib import ExitStack

import concourse.bass as bass
import concourse.bacc as bacc
import concourse.mybir as mybir
import concourse.tile as tile
from concourse.bass_utils import run_bass_kernel_spmd

F32 = mybir.dt.float32
BF16 = mybir.dt.bfloat16
FP8 = mybir.dt.float8e4
AF = mybir.ActivationFunctionType
ALU = mybir.AluOpType
AX = mybir.AxisListType
DR = mybir.MatmulPerfMode.DoubleRow

H, HD, NH, NKV = 4096, 128, 32, 8
L, V, S, I = 2, 32000, 1024, 11008
EPS, THETA = 1e-6, 10000.0
NC_ = 8          # cores
IPC = I // NC_   # 1376
IP = 1408        # padded intermediate per core = 11 * 128
IP2 = 1536       # fp8-pair-padded contraction for down proj = 12 * 128
VS = V // NC_    # 4000 vocab per core
NEG = -1e9
WS = 64.0        # fp8 weight scale (qkv, o, gate, down, lm_head)
US = 4.0         # fp8 weight scale for y=silu(g)*u (y*US must stay under 240)
LM_MAX = 16.0    # fixed logsumexp shift (|logit| << 16)
SC_MAX = 12.0    # fixed softmax shift for attention scores (|score| << 12)

bf16 = ml_dtypes.bfloat16
f8 = ml_dtypes.float8_e4m3

last_run_info = {}
_cache = {}


# ----------------------------------------------------------------- device --

def _norm_transpose(nc, small, ntmp, psum, h_ap, dst, ident_sb, uid,
                    nt_tag="mix", nt_bufs=2):
    """dst[:, k, :] (32 chunks of [128,128]) = normalized transpose of
    h_ap ([128 seq rows, 4096]). dst free dims must be (32, 128)."""
    ssq = small.tile([128, 1], F32, tag="nt_ssq", bufs=2, name=f"ssq_{uid}")
    # Square scratch output goes into dst (overwritten by the transpose after)
    nc.scalar.activation(dst, h_ap.rearrange("p (k m) -> p k m", k=32),
                         AF.Square, accum_out=ssq[:])
    var = small.tile([128, 1], F32, tag="nt_var", bufs=2, name=f"var_{uid}")
    nc.vector.tensor_scalar(var[:], ssq[:], 1.0 / H, EPS, op0=ALU.mult, op1=ALU.add)
    std = small.tile([128, 1], F32, tag="nt_std", bufs=2, name=f"std_{uid}")
    nc.scalar.sqrt(std[:], var[:])
    fac = small.tile([128, 1], F32, tag="nt_fac", bufs=2, name=f"fac_{uid}")
    nc.vector.reciprocal(fac[:], std[:])
    diag = ntmp.tile([128, 128], BF16, tag="nt_diag", bufs=2, name=f"diag_{uid}")
    nc.vector.tensor_scalar_mul(diag[:], ident_sb[:], fac[:])
    for kk in range(8):
        pnt = psum.tile([128, 512], F32, tag=nt_tag, bufs=nt_bufs,
                        name=f"pnt_{uid}_{kk}")
        for j in range(4):
            k = kk * 4 + j
            nc.tensor.matmul(pnt[:, j * 128:(j + 1) * 128],
                             h_ap[:, k * 128:(k + 1) * 128], diag[:],
                             start=True, stop=True)
        nc.any.tensor_copy(dst[:, kk * 4:(kk + 1) * 4, :],
                           pnt[:].rearrange("p (j m) -> p j m", j=4))


def _rope(nc, ntmp, ps, cos_ap, sf_ap, out, nheads, i):
    """out (bf16 [128, nheads*128]) = rope(ps); cos_ap/sf_ap are [128,128]."""
    n = nheads * 128
    t1 = ntmp.tile([128, 512], F32, tag="rope_t1", bufs=1, name=f"t1_{i}_{nheads}")
    t2 = ntmp.tile([128, 512], F32, tag="rope_t2", bufs=1, name=f"t2_{i}_{nheads}")
    for hh in range(nheads):
        b = hh * 128
        nc.vector.tensor_mul(t1[:, b:b + 128], ps[:, b:b + 128], cos_ap)
        nc.vector.tensor_mul(t2[:, b:b + 64], ps[:, b + 64:b + 128],
                             sf_ap[:, 0:64])
        nc.vector.tensor_mul(t2[:, b + 64:b + 128], ps[:, b:b + 64],
                             sf_ap[:, 64:128])
    nc.vector.tensor_add(out[:], t1[:, :n], t2[:, :n])


def build_nc():
    nc = bacc.Bacc("TRN2", target_bir_lowering=False, debug=False,
                   num_devices=NC_)

    din = {}
    def dram_in(name, shape, dtype=BF16):
        din[name] = nc.dram_tensor(name, shape, dtype, kind="ExternalInput")
        return din[name]

    h0_d = dram_in("h0", [S, H])
    cos1_d = dram_in("cos1", [S, 128])
    sf1_d = dram_in("sf1", [S, 128])
    ident_d = dram_in("ident", [128, 128])
    cmask_d = dram_in("cmask", [128, 128])
    ones_d = dram_in("ones", [128, 1])
    for l in range(L):
        dram_in(f"qkvw{l}", [128, 32, 768], FP8)
        dram_in(f"ow{l}", [128, 4, H], FP8)
        dram_in(f"gw{l}", [3, 8, 128, 4, 512], FP8)   # [nb, kp, p, j, n]
        dram_in(f"uw{l}", [3, 8, 128, 4, 512], FP8)
        dram_in(f"dw{l}", [8, 3, 128, 4, 512], FP8)   # [n, tp, p, j, n]
    lmw_d = dram_in("lmw", [8, 8, 128, 4, 500], FP8)  # [vb, kp, p, j, n]
    wsel_d = dram_in("wsel", [H, S])

    gsum_o = nc.dram_tensor("gsum_o", [128, 8], F32, kind="ExternalOutput")
    tlog_o = nc.dram_tensor("tlog_o", [1, S], F32, kind="ExternalOutput")

    rg = [list(range(NC_))]

    with tile.TileContext(nc) as tc:
        with (
            tc.tile_pool(name="pconst", bufs=1) as pconst,
            tc.tile_pool(name="psmall", bufs=1) as psmall,
            tc.tile_pool(name="pbridge", bufs=1) as pbridge,
            tc.tile_pool(name="pdram", bufs=1, space="DRAM") as pdram,
        ):
            ident_sb = pconst.tile([128, 128], BF16)
            cmask_sb = pconst.tile([128, 128], BF16)
            ones_sb = pconst.tile([128, 1], BF16)
            cos_sb = pconst.tile([128, 8, 128], BF16)
            sf_sb = pconst.tile([128, 8, 128], BF16)
            negSM = pconst.tile([128, 1], F32)
            nc.any.memset(negSM[:], -SC_MAX)
            nc.sync.dma_start(ident_sb[:], ident_d.ap())
            nc.sync.dma_start(cmask_sb[:], cmask_d.ap())
            nc.sync.dma_start(ones_sb[:], ones_d.ap())
            for i in range(8):
                nc.sync.dma_start(cos_sb[:, i, :], cos1_d.ap()[i * 128:(i + 1) * 128, :])
                nc.sync.dma_start(sf_sb[:, i, :], sf1_d.ap()[i * 128:(i + 1) * 128, :])

            xnt = [pbridge.tile([128, 32, 128], FP8, name=f"xnt_{j}")
                   for j in range(8)]

            hstack = ExitStack()
            phh = hstack.enter_context(tc.tile_pool(name="phh", bufs=1))
            h_sb = phh.tile([128, 8, H], BF16)
            for i in range(8):
                nc.sync.dma_start(h_sb[:, i, :], h0_d.ap()[i * 128:(i + 1) * 128, :])

            # quarter-grained AR buffers: [4 quarters][256, H]
            ar_ins, ar_outss, ar2_ins, ar2_outss = [], [], [], []
            for l in range(L):
                ar_ins.append(pdram.tile([S, H], BF16, name=f"ar_in_{l}"))
                ar_outss.append([pdram.tile([256, H], BF16, addr_space="Shared",
                                            name=f"ar_out_{l}_{q}")
                                 for q in range(4)])
                ar2_ins.append(pdram.tile([S, H], BF16, name=f"ar2_in_{l}"))
                ar2_outss.append([pdram.tile([256, H], BF16, addr_space="Shared",
                                             name=f"ar2_out_{l}_{q}")
                                  for q in range(4)])

            def prep(pool, psum, j, res_q, dst, uid, nt_tag="mix", nt_bufs=1):
                """h_sb[:,j] += AR-quarter residual; dst = norm-transpose."""
                if res_q is not None:
                    rt = pool.tile([128, H], BF16, tag="prep_rt", bufs=2,
                                   name=f"rt_{uid}")
                    nc.sync.dma_start(
                        rt[:], res_q[(j % 2) * 128:(j % 2 + 1) * 128, :])
                    nc.vector.tensor_add(h_sb[:, j, :], h_sb[:, j, :], rt[:])
                _norm_transpose(nc, psmall, pool, psum, h_sb[:, j, :], dst,
                                ident_sb, uid, nt_tag=nt_tag, nt_bufs=nt_bufs)

            xfstack = ExitStack()

            for l in range(L):
                # ======== attention: per-tile qkv -> heads -> o-proj ========
                with (
                    tc.tile_pool(name="pal", bufs=1) as pal,
                    tc.tile_pool(name="paps", bufs=1, space="PSUM") as paps,
                ):
                    kT_sb = pal.tile([128, S], BF16)
                    v_sb = pal.tile([128, 8, 128], BF16)
                    ar_in = ar_ins[l]
                    ar_outs = ar_outss[l]
                    wqkv_sb = pal.tile([128, 32, 768], FP8)
                    ow_sb = pal.tile([128, 4, H], FP8)
                    nc.sync.dma_start(wqkv_sb[:], din[f"qkvw{l}"].ap())
                    nc.sync.dma_start(ow_sb[:], din[f"ow{l}"].ap())
                    if l == 0:
                        for j in range(8):
                            prep(pal, paps, j, None, xnt[j], f"i{j}")
                    for i in range(8):
                        if l > 0 and i in (0, 1, 3, 4):
                            # prep the second-half attention tiles of this
                            # layer as their ar2 quarters land (q3 is late)
                            j = {0: 4, 1: 5, 3: 6, 4: 7}[i]
                            prep(pal, paps, j, ar2_outss[l - 1][j // 2],
                                 xnt[j], f"a{l}_{j}")
                        psq = paps.tile([128, 512], F32, tag="psq", bufs=1,
                                        name=f"psq_{l}_{i}")
                        pskv = paps.tile([128, 256], F32, tag="pskv", bufs=1,
                                         name=f"pskv_{l}_{i}")
                        for k in range(16):
                            nc.tensor.matmul(psq[:], xnt[i][:, 2 * k:2 * k + 2, :],
                                             wqkv_sb[:, 2 * k:2 * k + 2, 0:512],
                                             start=(k == 0), stop=(k == 15),
                                             perf_mode=DR)
                            nc.tensor.matmul(pskv[:], xnt[i][:, 2 * k:2 * k + 2, :],
                                             wqkv_sb[:, 2 * k:2 * k + 2, 512:768],
                                             start=(k == 0), stop=(k == 15),
                                             perf_mode=DR)
                        qT_sb = pal.tile([128, 4, 128], BF16, tag="qT",
                                         bufs=2, name=f"qT_{l}_{i}")
                        oT_sb = pal.tile([128, 4, 128], FP8, tag="oT",
                                         bufs=2, name=f"oT_{l}_{i}")
                        q_rot = pal.tile([128, 512], BF16, tag="q_rot", bufs=2,
                                         name=f"qr_{l}_{i}")
                        k_rot = pal.tile([128, 128], BF16, tag="k_rot", bufs=2,
                                         name=f"kr_{l}_{i}")
                        _rope(nc, pal, psq[:], cos_sb[:, i, :], sf_sb[:, i, :],
                              q_rot, 4, f"{l}_{i}")
                        _rope(nc, pal, pskv[:, 0:128], cos_sb[:, i, :],
                              sf_sb[:, i, :], k_rot, 1, f"{l}_{i}")
                        nc.vector.tensor_scalar_mul(v_sb[:, i, :],
                                                    pskv[:, 128:256], 1.0 / WS)
                        for hh in range(4):
                            ptr = paps.tile([128, 512], F32, tag="mix", bufs=1,
                                            name=f"ptrq_{l}_{i}_{hh}")
                            nc.tensor.matmul(ptr[:, :128], q_rot[:, hh * 128:(hh + 1) * 128],
                                             ident_sb[:], start=True, stop=True)
                            nc.any.tensor_copy(qT_sb[:, hh, :], ptr[:, :128])
                        ptrk = paps.tile([128, 512], F32, tag="mix", bufs=1,
                                         name=f"ptrk_{l}_{i}")
                        nc.tensor.matmul(ptrk[:, :128], k_rot[:], ident_sb[:],
                                         start=True, stop=True)
                        nc.any.tensor_copy(kT_sb[:, i * 128:(i + 1) * 128], ptrk[:, :128])
                        n2 = 128 * (i + 1)
                        for hh in range(4):
                            pss = paps.tile([128, 1024], F32, tag="pss", bufs=2,
                                            name=f"pss_{l}_{hh}_{i}")
                            lhs_q = qT_sb[:, hh, :]
                            c0 = 0
                            while c0 < n2 - 128:
                                N = min(512, n2 - 128 - c0)
                                nc.tensor.matmul(pss[:, c0:c0 + N], lhs_q,
                                                 kT_sb[:, c0:c0 + N],
                                                 start=True, stop=True)
                                c0 += N
                            nc.tensor.matmul(pss[:, n2 - 128:n2], lhs_q,
                                             kT_sb[:, n2 - 128:n2],
                                             start=True, stop=False)
                            nc.tensor.matmul(pss[:, n2 - 128:n2], ident_sb[:],
                                             cmask_sb[:], start=False, stop=True)
                            sume = psmall.tile([128, 1], F32, tag="sume", bufs=2,
                                               name=f"sume_{l}_{hh}_{i}")
                            exp_sb = pal.tile([128, 1024], BF16, tag="exp", bufs=2,
                                              name=f"exp_{l}_{hh}_{i}")
                            nc.scalar.activation(exp_sb[:, :n2], pss[:, :n2], AF.Exp,
                                                 bias=negSM[:], accum_out=sume[:])
                            rec = psmall.tile([128, 1], F32, tag="rec", bufs=2,
                                              name=f"rec_{l}_{hh}_{i}")
                            nc.vector.reciprocal(rec[:], sume[:])
                            diag_r = pal.tile([128, 128], BF16, tag="diag_r", bufs=2,
                                              name=f"diagr_{l}_{hh}_{i}")
                            nc.vector.tensor_scalar_mul(diag_r[:], ident_sb[:], rec[:])
                            atcol = pal.tile([128, 8, 128], BF16, tag="atcol", bufs=2,
                                             name=f"atcol_{l}_{hh}_{i}")
                            for j in range(i + 1):
                                pat = paps.tile([128, 512], F32, tag="mix", bufs=1,
                                                name=f"pat_{l}_{hh}_{i}_{j}")
                                nc.tensor.matmul(pat[:, :128], exp_sb[:, j * 128:(j + 1) * 128],
                                                 diag_r[:], start=True, stop=True)
                                nc.any.tensor_copy(atcol[:, j, :], pat[:, :128])
                            pso = paps.tile([128, 128], F32, tag="pso", bufs=1,
                                            name=f"pso_{l}_{hh}_{i}")
                            for j in range(i + 1):
                                nc.tensor.matmul(pso[:], v_sb[:, j, :], atcol[:, j, :],
                                                 start=(j == 0), stop=(j == i))
                            nc.any.tensor_copy(oT_sb[:, hh, :], pso[:])
                        ob = pal.tile([128, H], BF16, tag="ob", bufs=1,
                                      name=f"ob_{l}_{i}")
                        for n in range(8):
                            pps = paps.tile([128, 512], F32, tag="mix", bufs=1,
                                            name=f"pop_{l}_{i}_{n}")
                            for t in range(2):
                                nc.tensor.matmul(pps[:], oT_sb[:, 2 * t:2 * t + 2, :],
                                                 ow_sb[:, 2 * t:2 * t + 2,
                                                       n * 512:(n + 1) * 512],
                                                 start=(t == 0), stop=(t == 1),
                                                 perf_mode=DR)
                            nc.vector.tensor_scalar_mul(ob[:, n * 512:(n + 1) * 512],
                                                        pps[:], 1.0 / WS)
                        nc.sync.dma_start(ar_in[i * 128:(i + 1) * 128, :], ob[:])
                        if i % 2 == 1:
                            q = i // 2
                            nc.gpsimd.collective_compute(
                                "AllReduce", ALU.add, replica_groups=rg,
                                ins=[ar_in[q * 256:(q + 1) * 256, :].opt()],
                                outs=[ar_outs[q].opt()])
                        if i >= 4:
                            # prep MLP tiles 0..3 (attn AR quarters 0,1 ready)
                            j = i - 4
                            prep(pal, paps, j, ar_outs[j // 2], xnt[j],
                                 f"m{l}_{j}")
                    # prep MLP tiles 4,5 (quarter 2 posted after i==5)
                    for j in (4, 5):
                        prep(pal, paps, j, ar_outs[j // 2], xnt[j], f"m{l}_{j}")

                # ===== MLP: gate/up -> down in row-quarters -> AR2 ==========
                if l == L - 1:
                    pxf = xfstack.enter_context(tc.tile_pool(name="pxf", bufs=1))
                    xf_sb = pxf.tile([128, 32, S], FP8)
                with (
                    tc.tile_pool(name="pml", bufs=1) as pml,
                    tc.tile_pool(name="pmps", bufs=1, space="PSUM") as pmps,
                ):
                    ar2_in = ar2_ins[l]
                    ar2_outs = ar2_outss[l]

                    def prep_next(j):
                        """prep for the next phase: attention l+1 (into xnt)
                        or the final norm (into xf)."""
                        if l < L - 1:
                            prep(pml, pmps, j, ar2_outs[j // 2], xnt[j],
                                 f"a{l + 1}_{j}", nt_tag="mlpps", nt_bufs=4)
                        else:
                            prep(pml, pmps, j, ar2_outs[j // 2],
                                 xf_sb[:, :, j * 128:(j + 1) * 128],
                                 f"f{j}", nt_tag="mlpps", nt_bufs=4)

                    for ig in range(2):
                        with tc.tile_pool(name="pgu", bufs=1) as pgu:
                            if ig == 1:
                                # prep MLP tiles 6,7 (attn AR q3 long done)
                                for j in (6, 7):
                                    prep(pml, pmps, j, ar_outs[j // 2], xnt[j],
                                         f"m{l}_{j}", nt_tag="mlpps", nt_bufs=4)
                            yt_sb = pml.tile([128, 12, 512], FP8, tag="yt",
                                             bufs=2, name=f"yt_{l}_{ig}")
                            nc.vector.memset(yt_sb[:, 11, :], 0.0)
                            gu = {}
                            for wi, (wname, tag) in enumerate(
                                    ((f"gw{l}", "g"), (f"uw{l}", "u"))):
                                outs = [pgu.tile([128, IP], BF16, tag=tag, bufs=4,
                                                 name=f"{tag}_{l}_{ig}_{ii}")
                                        for ii in range(4)]
                                gu[tag] = outs
                                for nb in range(3):
                                    NB = 512 if nb < 2 else IP - 1024
                                    pg = [pmps.tile([128, 512], F32, tag="mlpps", bufs=4,
                                                    name=f"pg_{l}_{ig}_{tag}_{nb}_{ii}")
                                          for ii in range(4)]
                                    for kp in range(8):
                                        wt = pgu.tile([128, 4, 512], FP8, tag="wstream",
                                                      bufs=4,
                                                      name=f"wt_{l}_{ig}_{tag}_{nb}_{kp}")
                                        nc.sync.dma_start(wt[:], din[wname].ap()[nb, kp])
                                        for jp in range(2):
                                            k = kp * 4 + 2 * jp
                                            for ii in range(4):
                                                nc.tensor.matmul(
                                                    pg[ii][:, :NB],
                                                    xnt[ig * 4 + ii][:, k:k + 2, :],
                                                    wt[:, 2 * jp:2 * jp + 2, :NB],
                                                    start=(k == 0), stop=(k == 30),
                                                    perf_mode=DR)
                                    for ii in range(4):
                                        nc.any.tensor_copy(
                                            outs[ii][:, nb * 512:nb * 512 + NB],
                                            pg[ii][:, :NB])
                                    # interleave next-phase preps into ig=1
                                    if ig == 1 and wi == 0:
                                        prep_next(nb)          # tiles 0,1,2
                                    if ig == 1 and wi == 1 and nb == 0:
                                        prep_next(3)
                            for iq in range(2):
                                for ii in (iq * 2, iq * 2 + 1):
                                    i = ig * 4 + ii
                                    ysil = pgu.tile([128, IP], BF16, tag="ysil", bufs=2,
                                                    name=f"ysil_{l}_{i}")
                                    nc.scalar.activation(ysil[:], gu["g"][ii][:], AF.Silu,
                                                         scale=1.0 / WS)
                                    y = gu["u"][ii]
                                    nc.vector.tensor_mul(y[:], ysil[:], y[:])
                                    for tq in range(3):
                                        ts = [tq * 4 + j for j in range(4) if tq * 4 + j < 11]
                                        ptr = pmps.tile([128, 512], F32, tag="mlpps", bufs=4,
                                                        name=f"ytr_{l}_{i}_{tq}")
                                        for jj, t in enumerate(ts):
                                            nc.tensor.matmul(ptr[:, jj * 128:(jj + 1) * 128],
                                                             y[:, t * 128:(t + 1) * 128],
                                                             ident_sb[:], start=True, stop=True)
                                        nc.any.tensor_copy(
                                            yt_sb[:, ts[0]:ts[0] + len(ts),
                                                  ii * 128:(ii + 1) * 128],
                                            ptr[:, :len(ts) * 128].rearrange(
                                                "p (j m) -> p j m", j=len(ts)))
                                for n in range(8):
                                    pd = [pmps.tile([128, 512], F32, tag=f"pd{i2}", bufs=1,
                                                    name=f"pd_{l}_{ig}_{iq}_{n}_{i2}")
                                          for i2 in range(2)]
                                    for tp in range(3):
                                        dwt = pgu.tile([128, 4, 512], FP8, tag="dwstream",
                                                       bufs=4,
                                                       name=f"dwt_{l}_{ig}_{iq}_{n}_{tp}")
                                        nc.sync.dma_start(dwt[:], din[f"dw{l}"].ap()[n, tp])
                                        for jp in range(2):
                                            c = tp * 4 + 2 * jp
                                            for i2 in range(2):
                                                ii = iq * 2 + i2
                                                nc.tensor.matmul(
                                                    pd[i2][:],
                                                    yt_sb[:, c:c + 2,
                                                          ii * 128:(ii + 1) * 128],
                                                    dwt[:, 2 * jp:2 * jp + 2, :],
                                                    start=(c == 0), stop=(c == 10),
                                                    perf_mode=DR)
                                    for i2 in range(2):
                                        i = ig * 4 + iq * 2 + i2
                                        db = pgu.tile([128, 512], BF16, tag="db", bufs=2,
                                                      name=f"db_{l}_{ig}_{iq}_{n}_{i2}")
                                        nc.vector.tensor_scalar_mul(db[:], pd[i2][:],
                                                                    1.0 / (WS * US))
                                        nc.sync.dma_start(
                                            ar2_in[i * 128:(i + 1) * 128,
                                                   n * 512:(n + 1) * 512], db[:])
                                q = ig * 2 + iq
                                nc.gpsimd.collective_compute(
                                    "AllReduce", ALU.add, replica_groups=rg,
                                    ins=[ar2_in[q * 256:(q + 1) * 256, :].opt()],
                                    outs=[ar2_outs[q].opt()])
                    if l == L - 1:
                        # final xf tiles 4..7 (ar2 q2/q3 land late; short tail)
                        for j in (4, 5, 6, 7):
                            prep_next(j)

            # ==================== tlog + lm (online softmax) ================
            with (
                tc.tile_pool(name="ptl", bufs=1) as ptl,
                tc.tile_pool(name="ptps", bufs=1, space="PSUM") as ptps,
            ):
                pts = [ptps.tile([1, 512], F32, name=f"pt{h_}") for h_ in range(2)]
                for half in range(2):
                    for kp in range(8):
                        ws = ptl.tile([128, 4, 512], BF16, tag="wsel", bufs=2,
                                      name=f"ws_{half}_{kp}")
                        nc.sync.dma_start(
                            ws[:], wsel_d.ap()[kp * 512:(kp + 1) * 512,
                                               half * 512:(half + 1) * 512]
                            .rearrange("(j p) n -> p j n", p=128))
                        for jk in range(4):
                            k = kp * 4 + jk
                            tm = ptl.tile([128, 512], BF16, tag="tm", bufs=2,
                                          name=f"tm_{half}_{k}")
                            nc.vector.tensor_mul(
                                tm[:], xf_sb[:, k, half * 512:(half + 1) * 512],
                                ws[:, jk, :])
                            nc.tensor.matmul(pts[half][:], ones_sb[:], tm[:],
                                             start=(k == 0), stop=(k == 31))
                tl_sb = ptl.tile([1, S], F32)
                nc.any.tensor_copy(tl_sb[:, :512], pts[0][:])
                nc.any.tensor_copy(tl_sb[:, 512:], pts[1][:])
                nc.sync.dma_start(tlog_o.ap(), tl_sb[:])

            with (
                tc.tile_pool(name="plm", bufs=1) as plm,
                tc.tile_pool(name="plps", bufs=1, space="PSUM") as plps,
                tc.tile_pool(name="pld", bufs=1, space="DRAM") as pld,
            ):
                s_sb = plm.tile([128, 8], F32)
                nc.any.memset(s_sb[:], 0.0)
                negM = plm.tile([128, 1], F32)
                nc.any.memset(negM[:], -LM_MAX)
                for vb in range(8):
                    pl = [plps.tile([128, 500], F32, tag=f"pl{i}", bufs=1,
                                    name=f"pl_{vb}_{i}") for i in range(8)]
                    for kp in range(8):
                        lt = plm.tile([128, 4, 500], FP8, tag="lmw", bufs=4,
                                      name=f"lt_{vb}_{kp}")
                        nc.sync.dma_start(lt[:], lmw_d.ap()[vb, kp])
                        for jp in range(2):
                            k = kp * 4 + 2 * jp
                            for i in range(8):
                                nc.tensor.matmul(pl[i][:],
                                                 xf_sb[:, k:k + 2, i * 128:(i + 1) * 128],
                                                 lt[:, 2 * jp:2 * jp + 2, :],
                                                 start=(k == 0), stop=(k == 30),
                                                 perf_mode=DR)
                    for i in range(8):
                        se = psmall.tile([128, 1], F32, tag="se", bufs=2,
                                         name=f"se_{vb}_{i}")
                        scr = plm.tile([128, 500], BF16, tag="scr", bufs=2,
                                       name=f"scr_{vb}_{i}")
                        nc.scalar.activation(scr[:], pl[i][:], AF.Exp,
                                             bias=negM[:], scale=1.0 / WS,
                                             accum_out=se[:])
                        nc.vector.tensor_add(s_sb[:, i:i + 1], s_sb[:, i:i + 1],
                                             se[:])
                gs_in = pld.tile([128, 8], F32)
                gs_out = pld.tile([128, 8], F32, addr_space="Shared")
                nc.sync.dma_start(gs_in[:], s_sb[:])
                nc.gpsimd.collective_compute("AllReduce", ALU.add, replica_groups=rg,
                                             ins=[gs_in.opt()], outs=[gs_out.opt()])
                gsf_sb = plm.tile([128, 8], F32)
                nc.sync.dma_start(gsf_sb[:], gs_out[:])
                nc.sync.dma_start(gsum_o.ap(), gsf_sb[:])
            xfstack.close()
            hstack.close()

    nc.compile()
    return nc


# ------------------------------------------------------------------- host --

def _to_f8(x):
    return np.clip(x, -240.0, 240.0).astype(f8)


def host_prep(inputs):
    inp = {k: np.asarray(v) for k, v in inputs.items()}
    embed = inp["embed"].astype(np.float32)
    ids = inp["input_ids"].reshape(-1).astype(np.int64)
    labels = inp["labels"].reshape(-1).astype(np.int64)

    h = embed[ids]
    cw = inp["conv_w"].astype(np.float32)
    logit = h[:-1] @ cw[0, :H] + h[1:] @ cw[0, H:] + np.float32(inp["conv_b"][0])
    mask = logit > 0
    m = np.concatenate([mask, [False]])
    hn = np.where(m[:, None], 0.5 * (h + np.roll(h, -1, axis=0)), h)
    keep = np.concatenate([[True], ~mask])
    order = np.argsort(~keep, kind="stable")
    h0 = hn[order]
    lab = labels[order]
    valid_len = int(keep.sum())

    inv = 1.0 / (THETA ** (np.arange(0, HD, 2, dtype=np.float32) / HD))
    t = np.arange(S, dtype=np.float32)
    freqs = np.outer(t, inv)
    emb = np.concatenate([freqs, freqs], -1)
    cos, sin = np.cos(emb), np.sin(emb)
    sinflip = np.concatenate([-sin[:, :HD // 2], sin[:, HD // 2:]], -1)
    # rope constants absorb the 1/WS compensation for the fp8 q/k weights
    cos1 = (cos / WS).astype(bf16)
    sf1 = (sinflip / WS).astype(bf16)

    ident = np.eye(128, dtype=bf16)
    cmask = np.where(np.arange(128)[None, :] > np.arange(128)[:, None],
                     np.float32(NEG), np.float32(0)).astype(bf16)
    ones = np.ones((128, 1), dtype=bf16)

    ln1 = inp["ln1_w"].astype(np.float32)
    ln2 = inp["ln2_w"].astype(np.float32)
    normw = inp["norm_w"].astype(np.float32)
    qsc = np.float32(1.0 / np.sqrt(HD))
    lm_folded = normw[:, None] * inp["lm_head_w"].astype(np.float32)
    lm_q = _to_f8(lm_folded * WS)          # quantized once, reused for wsel
    tgt = np.concatenate([lab[1:], [0]]).astype(np.int64)
    wsel = np.ascontiguousarray(lm_q.astype(np.float32)[:, tgt] / WS).astype(bf16)

    common = dict(h0=h0.astype(bf16), cos1=cos1, sf1=sf1, ident=ident,
                  cmask=cmask, ones=ones, wsel=wsel)
    in_maps = []
    for c in range(NC_):
        mcore = dict(common)
        for l in range(L):
            qw = ln1[l][:, None] * inp["q_w"][l].astype(np.float32) * qsc * WS
            kw = ln1[l][:, None] * inp["k_w"][l].astype(np.float32) * WS
            vw = ln1[l][:, None] * inp["v_w"][l].astype(np.float32) * WS
            gw = ln2[l][:, None] * inp["gate_w"][l].astype(np.float32) * WS
            uw = ln2[l][:, None] * inp["up_w"][l].astype(np.float32) * US
            dw = inp["down_w"][l].astype(np.float32) * WS
            qkv = np.concatenate(
                [qw[:, c * 512:(c + 1) * 512],
                 kw[:, c * 128:(c + 1) * 128],
                 vw[:, c * 128:(c + 1) * 128]], 1)          # [H, 768]
            mcore[f"qkvw{l}"] = np.ascontiguousarray(
                _to_f8(qkv).reshape(32, 128, 768).transpose(1, 0, 2))
            ow = inp["o_w"][l][c * 512:(c + 1) * 512].astype(np.float32) * WS
            mcore[f"ow{l}"] = np.ascontiguousarray(
                _to_f8(ow).reshape(4, 128, H).transpose(1, 0, 2))
            gws = np.zeros((H, IP), np.float32)
            uws = np.zeros((H, IP), np.float32)
            dws = np.zeros((IP2, H), np.float32)
            gws[:, :IPC] = gw[:, c * IPC:(c + 1) * IPC]
            uws[:, :IPC] = uw[:, c * IPC:(c + 1) * IPC]
            dws[:IPC] = dw[c * IPC:(c + 1) * IPC]
            for wname, warr in ((f"gw{l}", gws), (f"uw{l}", uws)):
                out = np.zeros((3, 8, 128, 4, 512), np.float32)
                for nb in range(3):
                    NBc = 512 if nb < 2 else IP - 1024
                    blk = warr[:, nb * 512:nb * 512 + NBc]       # [H, NBc]
                    out[nb, :, :, :, :NBc] = blk.reshape(
                        8, 4, 128, NBc).transpose(0, 2, 1, 3)
                mcore[wname] = _to_f8(out)
            dout = np.zeros((8, 3, 128, 4, 512), np.float32)
            for n in range(8):
                blk = dws[:, n * 512:(n + 1) * 512]              # [IP2, 512]
                dout[n] = blk.reshape(3, 4, 128, 512).transpose(0, 2, 1, 3)
            mcore[f"dw{l}"] = _to_f8(dout)
        lmc = lm_q[:, c * VS:(c + 1) * VS].astype(np.float32)    # [H, 4000]
        lout = np.zeros((8, 8, 128, 4, 500), np.float32)
        for vb in range(8):
            blk = lmc[:, vb * 500:(vb + 1) * 500]                # [H, 500]
            lout[vb] = blk.reshape(8, 4, 128, 500).transpose(0, 2, 1, 3)
        mcore["lmw"] = _to_f8(lout)
        in_maps.append(mcore)

    return in_maps, valid_len


def kernel(**inputs) -> np.ndarray:
    in_maps, valid_len = host_prep(inputs)
    if "nc" not in _cache:
        _cache["nc"] = build_nc()
    nc = _cache["nc"]
    res = run_bass_kernel_spmd(nc, in_maps, list(range(NC_)),
                               **last_run_info.get("run_kwargs", {}))
    last_run_info["res"] = res
    out = res.results[0]
    gsum = out["gsum_o"].transpose(1, 0).reshape(S).astype(np.float64)
    tlog = out["tlog_o"].reshape(S).astype(np.float64)
    ce = LM_MAX + np.log(gsum) - tlog
    w = (np.arange(S - 1) < valid_len - 1).astype(np.float64)
    loss = (ce[:S - 1] * w).sum() / w.sum()
    return np.float32(loss)


# revision 24
# speedup vs baseline: 1.2699x; 1.2699x over previous
"""Trainium2 Bass kernel for nn_Decoder_20486994002617.  v3.

8-core tensor-parallel 2-layer llama-style decoder with ragged token-merge
(handled on host), returning the masked-mean cross-entropy loss.

v2: fp8e4 DoubleRow for qkv / o / gate / up / down / lm_head, weights
pre-scaled (x64, up-proj x4) into e4m3 range, compensation folded into rope
constants and scaled PSUM->SBUF copies; host pre-chunks weights into SBUF
tile layout so streaming DMAs are contiguous.

v3 (latency restructure):
  - AllReduces quartered (256 rows each) and posted as soon as their rows
    are ready, so consumers never wait on a half-sequence collective.
  - A persistent "bridge" array xnt[0..7] holds the normalized transposed
    activations; the residual-add + rmsnorm + transpose for each phase is
    emitted inside the *previous* phase's instruction stream (prep), so
    GEMMs start immediately at phase entry.
  - lm head uses a fixed-max (M=16) online softmax: no logits storage, no
    max AllReduce, no DRAM spill of h.
  - tlog runs in sequence halves so it can start before the last xf tiles.
"""
import numpy as np
import ml_dtypes

from contextlib import ExitStack

import concourse.bass as bass
import concourse.bacc as bacc
import concourse.mybir as mybir
import concourse.tile as tile
from concourse.bass_utils import run_bass_kernel_spmd

F32 = mybir.dt.float32
BF16 = mybir.dt.bfloat16
FP8 = mybir.dt.float8e4
AF = mybir.ActivationFunctionType
ALU = mybir.AluOpType
AX = mybir.AxisListType
DR = mybir.MatmulPerfMode.DoubleRow

H, HD, NH, NKV = 4096, 128, 32, 8
L, V, S, I = 2, 32000, 1024, 11008
EPS, THETA = 1e-6, 10000.0
NC_ = 8          # cores
IPC = I // NC_   # 1376
IP = 1408        # padded intermediate per core = 11 * 128
IP2 = 1536       # fp8-pair-padded contraction for down proj = 12 * 128
VS = V // NC_    # 4000 vocab per core
NEG = -1e9
WS = 64.0        # fp8 weight scale (qkv, o, gate, down, lm_head)
US = 4.0         # fp8 weight scale for y=silu(g)*u (y*US must stay under 240)
LM_MAX = 16.0    # fixed logsumexp shift (|logit| << 16)
SC_MAX = 12.0    # fixed softmax shift for attention scores (|score| << 12)

bf16 = ml_dtypes.bfloat16
f8 = ml_dtypes.float8_e4m3

last_run_info = {}
_cache = {}


# ----------------------------------------------------------------- device --

def _norm_transpose(nc, small, ntmp, psum, h_ap, dst, ident_sb, uid,
                    nt_tag="mix", nt_bufs=2):
    """dst[:, k, :] (32 chunks of [128,128]) = normalized transpose of
    h_ap ([128 seq rows, 4096]). dst free dims must be (32, 128)."""
    ssq = small.tile([128, 1], F32, tag="nt_ssq", bufs=2, name=f"ssq_{uid}")
    # Square scratch output goes into dst (overwritten by the transpose after)
    nc.scalar.activation(dst, h_ap.rearrange("p (k m) -> p k m", k=32),
                         AF.Square, accum_out=ssq[:])
    var = small.tile([128, 1], F32, tag="nt_var", bufs=2, name=f"var_{uid}")
    nc.vector.tensor_scalar(var[:], ssq[:], 1.0 / H, EPS, op0=ALU.mult, op1=ALU.add)
    std = small.tile([128, 1], F32, tag="nt_std", bufs=2, name=f"std_{uid}")
    nc.scalar.sqrt(std[:], var[:])
    fac = small.tile([128, 1], F32, tag="nt_fac", bufs=2, name=f"fac_{uid}")
    nc.vector.reciprocal(fac[:], std[:])
    diag = ntmp.tile([128, 128], BF16, tag="nt_diag", bufs=2, name=f"diag_{uid}")
    nc.vector.tensor_scalar_mul(diag[:], ident_sb[:], fac[:])
    for kk in range(8):
        pnt = psum.tile([128, 512], F32, tag=nt_tag, bufs=nt_bufs,
                        name=f"pnt_{uid}_{kk}")
        for j in range(4):
            k = kk * 4 + j
            nc.tensor.matmul(pnt[:, j * 128:(j + 1) * 128],
                             h_ap[:, k * 128:(k + 1) * 128], diag[:],
                             start=True, stop=True)
        nc.any.tensor_copy(dst[:, kk * 4:(kk + 1) * 4, :],
                           pnt[:].rearrange("p (j m) -> p j m", j=4))


def _rope(nc, ntmp, ps, cos_ap, sf_ap, out, nheads, i):
    """out (bf16 [128, nheads*128]) = rope(ps); cos_ap/sf_ap are [128,128]."""
    n = nheads * 128
    t1 = ntmp.tile([128, 512], F32, tag="rope_t1", bufs=1, name=f"t1_{i}_{nheads}")
    t2 = ntmp.tile([128, 512], F32, tag="rope_t2", bufs=1, name=f"t2_{i}_{nheads}")
    for hh in range(nheads):
        b = hh * 128
        nc.vector.tensor_mul(t1[:, b:b + 128], ps[:, b:b + 128], cos_ap)
        nc.vector.tensor_mul(t2[:, b:b + 64], ps[:, b + 64:b + 128],
                             sf_ap[:, 0:64])
        nc.vector.tensor_mul(t2[:, b + 64:b + 128], ps[:, b:b + 64],
                             sf_ap[:, 64:128])
    nc.vector.tensor_add(out[:], t1[:, :n], t2[:, :n])


def build_nc():
    nc = bacc.Bacc("TRN2", target_bir_lowering=False, debug=False,
                   num_devices=NC_)

    din = {}
    def dram_in(name, shape, dtype=BF16):
        din[name] = nc.dram_tensor(name, shape, dtype, kind="ExternalInput")
        return din[name]

    h0_d = dram_in("h0", [S, H])
    cos1_d = dram_in("cos1", [S, 128])
    sf1_d = dram_in("sf1", [S, 128])
    ident_d = dram_in("ident", [128, 128])
    cmask_d = dram_in("cmask", [128, 128])
    ones_d = dram_in("ones", [128, 1])
    for l in range(L):
        dram_in(f"qkvw{l}", [128, 32, 768], FP8)
        dram_in(f"ow{l}", [128, 4, H], FP8)
        dram_in(f"gw{l}", [3, 8, 128, 4, 512], FP8)   # [nb, kp, p, j, n]
        dram_in(f"uw{l}", [3, 8, 128, 4, 512], FP8)
        dram_in(f"dw{l}", [8, 3, 128, 4, 512], FP8)   # [n, tp, p, j, n]
    lmw_d = dram_in("lmw", [8, 8, 128, 4, 500], FP8)  # [vb, kp, p, j, n]
    wsel_d = dram_in("wsel", [H, S])

    gsum_o = nc.dram_tensor("gsum_o", [128, 8], F32, kind="ExternalOutput")
    tlog_o = nc.dram_tensor("tlog_o", [1, S], F32, kind="ExternalOutput")

    rg = [list(range(NC_))]

    with tile.TileContext(nc) as tc:
        with (
            tc.tile_pool(name="pconst", bufs=1) as pconst,
            tc.tile_pool(name="psmall", bufs=1) as psmall,
            tc.tile_pool(name="pbridge", bufs=1) as pbridge,
            tc.tile_pool(name="pdram", bufs=1, space="DRAM") as pdram,
        ):
            ident_sb = pconst.tile([128, 128], BF16)
            cmask_sb = pconst.tile([128, 128], BF16)
            ones_sb = pconst.tile([128, 1], BF16)
            cos_sb = pconst.tile([128, 8, 128], BF16)
            sf_sb = pconst.tile([128, 8, 128], BF16)
            negSM = pconst.tile([128, 1], F32)
            nc.any.memset(negSM[:], -SC_MAX)
            nc.sync.dma_start(ident_sb[:], ident_d.ap())
            nc.sync.dma_start(cmask_sb[:], cmask_d.ap())
            nc.sync.dma_start(ones_sb[:], ones_d.ap())
            for i in range(8):
                nc.sync.dma_start(cos_sb[:, i, :], cos1_d.ap()[i * 128:(i + 1) * 128, :])
                nc.sync.dma_start(sf_sb[:, i, :], sf1_d.ap()[i * 128:(i + 1) * 128, :])

            xnt = [pbridge.tile([128, 32, 128], FP8, name=f"xnt_{j}")
                   for j in range(8)]

            hstack = ExitStack()
            phh = hstack.enter_context(tc.tile_pool(name="phh", bufs=1))
            h_sb = phh.tile([128, 8, H], BF16)
            for i in range(8):
                nc.sync.dma_start(h_sb[:, i, :], h0_d.ap()[i * 128:(i + 1) * 128, :])

            # quarter-grained AR buffers: [4 quarters][256, H]
            ar_ins, ar_outss, ar2_ins, ar2_outss = [], [], [], []
            for l in range(L):
                ar_ins.append(pdram.tile([S, H], BF16, name=f"ar_in_{l}"))
                ar_outss.append([pdram.tile([256, H], BF16, addr_space="Shared",
                                            name=f"ar_out_{l}_{q}")
                                 for q in range(4)])
                ar2_ins.append(pdram.tile([S, H], BF16, name=f"ar2_in_{l}"))
                ar2_outss.append([pdram.tile([256, H], BF16, addr_space="Shared",
                                             name=f"ar2_out_{l}_{q}")
                                  for q in range(4)])

            def prep(pool, psum, j, res_q, dst, uid, nt_tag="mix", nt_bufs=2):
                """h_sb[:,j] += AR-quarter residual; dst = norm-transpose."""
                if res_q is not None:
                    rt = pool.tile([128, H], BF16, tag="prep_rt", bufs=2,
                                   name=f"rt_{uid}")
                    nc.sync.dma_start(
                        rt[:], res_q[(j % 2) * 128:(j % 2 + 1) * 128, :])
                    nc.vector.tensor_add(h_sb[:, j, :], h_sb[:, j, :], rt[:])
                _norm_transpose(nc, psmall, pool, psum, h_sb[:, j, :], dst,
                                ident_sb, uid, nt_tag=nt_tag, nt_bufs=nt_bufs)

            xfstack = ExitStack()

            for l in range(L):
                # ======== attention: per-tile qkv -> heads -> o-proj ========
                with (
                    tc.tile_pool(name="pal", bufs=1) as pal,
                    tc.tile_pool(name="paps", bufs=1, space="PSUM") as paps,
                ):
                    kT_sb = pal.tile([128, S], BF16)
                    v_sb = pal.tile([128, 8, 128], BF16)
                    ar_in = ar_ins[l]
                    ar_outs = ar_outss[l]
                    wqkv_sb = pal.tile([128, 32, 768], FP8)
                    ow_sb = pal.tile([128, 4, H], FP8)
                    nc.sync.dma_start(wqkv_sb[:], din[f"qkvw{l}"].ap())
                    nc.sync.dma_start(ow_sb[:], din[f"ow{l}"].ap())
                    if l == 0:
                        for j in range(8):
                            prep(pal, paps, j, None, xnt[j], f"i{j}")
                    for i in range(8):
                        if l > 0 and i in (0, 1, 3, 4):
                            # prep the second-half attention tiles of this
                            # layer as their ar2 quarters land (q3 is late)
                            j = {0: 4, 1: 5, 3: 6, 4: 7}[i]
                            prep(pal, paps, j, ar2_outss[l - 1][j // 2],
                                 xnt[j], f"a{l}_{j}")
                        psq = paps.tile([128, 512], F32, tag="psq", bufs=1,
                                        name=f"psq_{l}_{i}")
                        pskv = paps.tile([128, 256], F32, tag="pskv", bufs=1,
                                         name=f"pskv_{l}_{i}")
                        for k in range(16):
                            nc.tensor.matmul(psq[:], xnt[i][:, 2 * k:2 * k + 2, :],
                                             wqkv_sb[:, 2 * k:2 * k + 2, 0:512],
                                             start=(k == 0), stop=(k == 15),
                                             perf_mode=DR)
                            nc.tensor.matmul(pskv[:], xnt[i][:, 2 * k:2 * k + 2, :],
                                             wqkv_sb[:, 2 * k:2 * k + 2, 512:768],
                                             start=(k == 0), stop=(k == 15),
                                             perf_mode=DR)
                        qT_sb = pal.tile([128, 4, 128], BF16, tag="qT",
                                         bufs=2, name=f"qT_{l}_{i}")
                        oT_sb = pal.tile([128, 4, 128], FP8, tag="oT",
                                         bufs=2, name=f"oT_{l}_{i}")
                        q_rot = pal.tile([128, 512], BF16, tag="q_rot", bufs=2,
                                         name=f"qr_{l}_{i}")
                        k_rot = pal.tile([128, 128], BF16, tag="k_rot", bufs=2,
                                         name=f"kr_{l}_{i}")
                        _rope(nc, pal, psq[:], cos_sb[:, i, :], sf_sb[:, i, :],
                              q_rot, 4, f"{l}_{i}")
                        _rope(nc, pal, pskv[:, 0:128], cos_sb[:, i, :],
                              sf_sb[:, i, :], k_rot, 1, f"{l}_{i}")
                        nc.vector.tensor_scalar_mul(v_sb[:, i, :],
                                                    pskv[:, 128:256], 1.0 / WS)
                        ptr = paps.tile([128, 512], F32, tag="mix", bufs=2,
                                        name=f"ptrq_{l}_{i}")
                        for hh in range(4):
                            nc.tensor.matmul(ptr[:, hh * 128:(hh + 1) * 128],
                                             q_rot[:, hh * 128:(hh + 1) * 128],
                                             ident_sb[:], start=True, stop=True)
                        nc.any.tensor_copy(qT_sb[:],
                                           ptr[:].rearrange("p (j m) -> p j m", j=4))
                        ptrk = paps.tile([128, 512], F32, tag="mix", bufs=2,
                                         name=f"ptrk_{l}_{i}")
                        nc.tensor.matmul(ptrk[:, :128], k_rot[:], ident_sb[:],
                                         start=True, stop=True)
                        nc.any.tensor_copy(kT_sb[:, i * 128:(i + 1) * 128], ptrk[:, :128])
                        n2 = 128 * (i + 1)
                        for hh in range(4):
                            pss = paps.tile([128, 1024], F32, tag="pss", bufs=1,
                                            name=f"pss_{l}_{hh}_{i}")
                            lhs_q = qT_sb[:, hh, :]
                            c0 = 0
                            while c0 < n2 - 128:
                                N = min(512, n2 - 128 - c0)
                                nc.tensor.matmul(pss[:, c0:c0 + N], lhs_q,
                                                 kT_sb[:, c0:c0 + N],
                                                 start=True, stop=True)
                                c0 += N
                            nc.tensor.matmul(pss[:, n2 - 128:n2], lhs_q,
                                             kT_sb[:, n2 - 128:n2],
                                             start=True, stop=False)
                            nc.tensor.matmul(pss[:, n2 - 128:n2], ident_sb[:],
                                             cmask_sb[:], start=False, stop=True)
                            sume = psmall.tile([128, 1], F32, tag="sume", bufs=2,
                                               name=f"sume_{l}_{hh}_{i}")
                            exp_sb = pal.tile([128, 1024], BF16, tag="exp", bufs=2,
                                              name=f"exp_{l}_{hh}_{i}")
                            nc.scalar.activation(exp_sb[:, :n2], pss[:, :n2], AF.Exp,
                                                 bias=negSM[:], accum_out=sume[:])
                            rec = psmall.tile([128, 1], F32, tag="rec", bufs=2,
                                              name=f"rec_{l}_{hh}_{i}")
                            nc.vector.reciprocal(rec[:], sume[:])
                            diag_r = pal.tile([128, 128], BF16, tag="diag_r", bufs=2,
                                              name=f"diagr_{l}_{hh}_{i}")
                            nc.vector.tensor_scalar_mul(diag_r[:], ident_sb[:], rec[:])
                            atcol = pal.tile([128, 8, 128], BF16, tag="atcol", bufs=2,
                                             name=f"atcol_{l}_{hh}_{i}")
                            for jg in range(0, i + 1, 4):
                                ng = min(4, i + 1 - jg)
                                pat = paps.tile([128, 512], F32, tag="mix", bufs=2,
                                                name=f"pat_{l}_{hh}_{i}_{jg}")
                                for jj in range(ng):
                                    j = jg + jj
                                    nc.tensor.matmul(pat[:, jj * 128:(jj + 1) * 128],
                                                     exp_sb[:, j * 128:(j + 1) * 128],
                                                     diag_r[:], start=True, stop=True)
                                nc.any.tensor_copy(
                                    atcol[:, jg:jg + ng, :],
                                    pat[:, :ng * 128].rearrange("p (j m) -> p j m", j=ng))
                            pso = paps.tile([128, 128], F32, tag="pso", bufs=2,
                                            name=f"pso_{l}_{hh}_{i}")
                            for j in range(i + 1):
                                nc.tensor.matmul(pso[:], v_sb[:, j, :], atcol[:, j, :],
                                                 start=(j == 0), stop=(j == i))
                            nc.any.tensor_copy(oT_sb[:, hh, :], pso[:])
                        ob = pal.tile([128, H], BF16, tag="ob", bufs=1,
                                      name=f"ob_{l}_{i}")
                        for n in range(8):
                            pps = paps.tile([128, 512], F32, tag="mix", bufs=2,
                                            name=f"pop_{l}_{i}_{n}")
                            for t in range(2):
                                nc.tensor.matmul(pps[:], oT_sb[:, 2 * t:2 * t + 2, :],
                                                 ow_sb[:, 2 * t:2 * t + 2,
                                                       n * 512:(n + 1) * 512],
                                                 start=(t == 0), stop=(t == 1),
                                                 perf_mode=DR)
                            nc.vector.tensor_scalar_mul(ob[:, n * 512:(n + 1) * 512],
                                                        pps[:], 1.0 / WS)
                        nc.sync.dma_start(ar_in[i * 128:(i + 1) * 128, :], ob[:])
                        if i % 2 == 1:
                            q = i // 2
                            nc.gpsimd.collective_compute(
                                "AllReduce", ALU.add, replica_groups=rg,
                                ins=[ar_in[q * 256:(q + 1) * 256, :].opt()],
                                outs=[ar_outs[q].opt()])
                        if i >= 4:
                            # prep MLP tiles 0..3 (attn AR quarters 0,1 ready)
                            j = i - 4
                            prep(pal, paps, j, ar_outs[j // 2], xnt[j],
                                 f"m{l}_{j}")
                    # prep MLP tiles 4,5 (quarter 2 posted after i==5)
                    for j in (4, 5):
                        prep(pal, paps, j, ar_outs[j // 2], xnt[j], f"m{l}_{j}")

                # ===== MLP: gate/up -> down in row-quarters -> AR2 ==========
                if l == L - 1:
                    pxf = xfstack.enter_context(tc.tile_pool(name="pxf", bufs=1))
                    xf_sb = pxf.tile([128, 32, S], FP8)
                with (
                    tc.tile_pool(name="pml", bufs=1) as pml,
                    tc.tile_pool(name="pmps", bufs=1, space="PSUM") as pmps,
                ):
                    ar2_in = ar2_ins[l]
                    ar2_outs = ar2_outss[l]

                    def prep_next(j):
                        """prep for the next phase: attention l+1 (into xnt)
                        or the final norm (into xf)."""
                        if l < L - 1:
                            prep(pml, pmps, j, ar2_outs[j // 2], xnt[j],
                                 f"a{l + 1}_{j}", nt_tag="mlpps", nt_bufs=4)
                        else:
                            prep(pml, pmps, j, ar2_outs[j // 2],
                                 xf_sb[:, :, j * 128:(j + 1) * 128],
                                 f"f{j}", nt_tag="mlpps", nt_bufs=4)

                    for ig in range(2):
                        with tc.tile_pool(name="pgu", bufs=1) as pgu:
                            if ig == 1:
                                # prep MLP tiles 6,7 (attn AR q3 long done)
                                for j in (6, 7):
                                    prep(pml, pmps, j, ar_outs[j // 2], xnt[j],
                                         f"m{l}_{j}", nt_tag="mlpps", nt_bufs=4)
                            yt_sb = pml.tile([128, 12, 512], FP8, tag="yt",
                                             bufs=2, name=f"yt_{l}_{ig}")
                            nc.vector.memset(yt_sb[:, 11, :], 0.0)
                            gu = {}
                            for wi, (wname, tag) in enumerate(
                                    ((f"gw{l}", "g"), (f"uw{l}", "u"))):
                                outs = [pgu.tile([128, IP], BF16, tag=tag, bufs=4,
                                                 name=f"{tag}_{l}_{ig}_{ii}")
                                        for ii in range(4)]
                                gu[tag] = outs
                                for nb in range(3):
                                    NB = 512 if nb < 2 else IP - 1024
                                    pg = [pmps.tile([128, 512], F32, tag="mlpps", bufs=4,
                                                    name=f"pg_{l}_{ig}_{tag}_{nb}_{ii}")
                                          for ii in range(4)]
                                    for kp in range(8):
                                        wt = pgu.tile([128, 4, 512], FP8, tag="wstream",
                                                      bufs=4,
                                                      name=f"wt_{l}_{ig}_{tag}_{nb}_{kp}")
                                        nc.sync.dma_start(wt[:], din[wname].ap()[nb, kp])
                                        for jp in range(2):
                                            k = kp * 4 + 2 * jp
                                            for ii in range(4):
                                                nc.tensor.matmul(
                                                    pg[ii][:, :NB],
                                                    xnt[ig * 4 + ii][:, k:k + 2, :],
                                                    wt[:, 2 * jp:2 * jp + 2, :NB],
                                                    start=(k == 0), stop=(k == 30),
                                                    perf_mode=DR)
                                    for ii in range(4):
                                        nc.any.tensor_copy(
                                            outs[ii][:, nb * 512:nb * 512 + NB],
                                            pg[ii][:, :NB])
                                    # interleave next-phase preps into ig=1
                                    if ig == 1 and wi == 0:
                                        prep_next(nb)          # tiles 0,1,2
                                    if ig == 1 and wi == 1 and nb == 0:
                                        prep_next(3)
                            for iq in range(2):
                                for ii in (iq * 2, iq * 2 + 1):
                                    i = ig * 4 + ii
                                    ysil = pgu.tile([128, IP], BF16, tag="ysil", bufs=2,
                                                    name=f"ysil_{l}_{i}")
                                    nc.scalar.activation(ysil[:], gu["g"][ii][:], AF.Silu,
                                                         scale=1.0 / WS)
                                    y = gu["u"][ii]
                                    nc.vector.tensor_mul(y[:], ysil[:], y[:])
                                    for tq in range(3):
                                        ts = [tq * 4 + j for j in range(4) if tq * 4 + j < 11]
                                        ptr = pmps.tile([128, 512], F32, tag="mlpps", bufs=4,
                                                        name=f"ytr_{l}_{i}_{tq}")
                                        for jj, t in enumerate(ts):
                                            nc.tensor.matmul(ptr[:, jj * 128:(jj + 1) * 128],
                                                             y[:, t * 128:(t + 1) * 128],
                                                             ident_sb[:], start=True, stop=True)
                                        nc.any.tensor_copy(
                                            yt_sb[:, ts[0]:ts[0] + len(ts),
                                                  ii * 128:(ii + 1) * 128],
                                            ptr[:, :len(ts) * 128].rearrange(
                                                "p (j m) -> p j m", j=len(ts)))
                                for n in range(8):
                                    pd = [pmps.tile([128, 512], F32, tag=f"pd{i2}", bufs=1,
                                                    name=f"pd_{l}_{ig}_{iq}_{n}_{i2}")
                                          for i2 in range(2)]
                                    for tp in range(3):
                                        dwt = pgu.tile([128, 4, 512], FP8, tag="dwstream",
                                                       bufs=4,
                                                       name=f"dwt_{l}_{ig}_{iq}_{n}_{tp}")
                                        nc.sync.dma_start(dwt[:], din[f"dw{l}"].ap()[n, tp])
                                        for jp in range(2):
                                            c = tp * 4 + 2 * jp
                                            for i2 in range(2):
                                                ii = iq * 2 + i2
                                                nc.tensor.matmul(
                                                    pd[i2][:],
                                                    yt_sb[:, c:c + 2,
                                                          ii * 128:(ii + 1) * 128],
                                                    dwt[:, 2 * jp:2 * jp + 2, :],
                                                    start=(c == 0), stop=(c == 10),
                                                    perf_mode=DR)
                                    for i2 in range(2):
                                        i = ig * 4 + iq * 2 + i2
                                        db = pgu.tile([128, 512], BF16, tag="db", bufs=2,
                                                      name=f"db_{l}_{ig}_{iq}_{n}_{i2}")
                                        nc.vector.tensor_scalar_mul(db[:], pd[i2][:],
                                                                    1.0 / (WS * US))
                                        nc.sync.dma_start(
                                            ar2_in[i * 128:(i + 1) * 128,
                                                   n * 512:(n + 1) * 512], db[:])
                                q = ig * 2 + iq
                                nc.gpsimd.collective_compute(
                                    "AllReduce", ALU.add, replica_groups=rg,
                                    ins=[ar2_in[q * 256:(q + 1) * 256, :].opt()],
                                    outs=[ar2_outs[q].opt()])
                    if l == L - 1:
                        # final xf tiles 4..7 (ar2 q2/q3 land late; short tail)
                        for j in (4, 5, 6, 7):
                            prep_next(j)

            # ==================== tlog + lm (online softmax) ================
            with (
                tc.tile_pool(name="ptl", bufs=1) as ptl,
                tc.tile_pool(name="ptps", bufs=1, space="PSUM") as ptps,
            ):
                pts = [ptps.tile([1, 512], F32, name=f"pt{h_}") for h_ in range(2)]
                for half in range(2):
                    for kp in range(8):
                        ws = ptl.tile([128, 4, 512], BF16, tag="wsel", bufs=2,
                                      name=f"ws_{half}_{kp}")
                        nc.sync.dma_start(
                            ws[:], wsel_d.ap()[kp * 512:(kp + 1) * 512,
                                               half * 512:(half + 1) * 512]
                            .rearrange("(j p) n -> p j n", p=128))
                        for jk in range(4):
                            k = kp * 4 + jk
                            tm = ptl.tile([128, 512], BF16, tag="tm", bufs=2,
                                          name=f"tm_{half}_{k}")
                            nc.vector.tensor_mul(
                                tm[:], xf_sb[:, k, half * 512:(half + 1) * 512],
                                ws[:, jk, :])
                            nc.tensor.matmul(pts[half][:], ones_sb[:], tm[:],
                                             start=(k == 0), stop=(k == 31))
                tl_sb = ptl.tile([1, S], F32)
                nc.any.tensor_copy(tl_sb[:, :512], pts[0][:])
                nc.any.tensor_copy(tl_sb[:, 512:], pts[1][:])
                nc.sync.dma_start(tlog_o.ap(), tl_sb[:])

            with (
                tc.tile_pool(name="plm", bufs=1) as plm,
                tc.tile_pool(name="plps", bufs=1, space="PSUM") as plps,
                tc.tile_pool(name="pld", bufs=1, space="DRAM") as pld,
            ):
                s_sb = plm.tile([128, 8], F32)
                nc.any.memset(s_sb[:], 0.0)
                negM = plm.tile([128, 1], F32)
                nc.any.memset(negM[:], -LM_MAX)
                for vb in range(8):
                    pl = [plps.tile([128, 500], F32, tag=f"pl{i}", bufs=1,
                                    name=f"pl_{vb}_{i}") for i in range(8)]
                    for kp in range(8):
                        lt = plm.tile([128, 4, 500], FP8, tag="lmw", bufs=4,
                                      name=f"lt_{vb}_{kp}")
                        nc.sync.dma_start(lt[:], lmw_d.ap()[vb, kp])
                        for jp in range(2):
                            k = kp * 4 + 2 * jp
                            for i in range(8):
                                nc.tensor.matmul(pl[i][:],
                                                 xf_sb[:, k:k + 2, i * 128:(i + 1) * 128],
                                                 lt[:, 2 * jp:2 * jp + 2, :],
                                                 start=(k == 0), stop=(k == 30),
                                                 perf_mode=DR)
                    for i in range(8):
                        se = psmall.tile([128, 1], F32, tag="se", bufs=2,
                                         name=f"se_{vb}_{i}")
                        scr = plm.tile([128, 500], BF16, tag="scr", bufs=2,
                                       name=f"scr_{vb}_{i}")
                        nc.scalar.activation(scr[:], pl[i][:], AF.Exp,
                                             bias=negM[:], scale=1.0 / WS,
                                             accum_out=se[:])
                        nc.vector.tensor_add(s_sb[:, i:i + 1], s_sb[:, i:i + 1],
                                             se[:])
                gs_in = pld.tile([128, 8], F32)
                gs_out = pld.tile([128, 8], F32, addr_space="Shared")
                nc.sync.dma_start(gs_in[:], s_sb[:])
                nc.gpsimd.collective_compute("AllReduce", ALU.add, replica_groups=rg,
                                             ins=[gs_in.opt()], outs=[gs_out.opt()])
                gsf_sb = plm.tile([128, 8], F32)
                nc.sync.dma_start(gsf_sb[:], gs_out[:])
                nc.sync.dma_start(gsum_o.ap(), gsf_sb[:])
            xfstack.close()
            hstack.close()

    nc.compile()
    return nc


# ------------------------------------------------------------------- host --

def _to_f8(x):
    return np.clip(x, -240.0, 240.0).astype(f8)


def host_prep(inputs):
    inp = {k: np.asarray(v) for k, v in inputs.items()}
    embed = inp["embed"].astype(np.float32)
    ids = inp["input_ids"].reshape(-1).astype(np.int64)
    labels = inp["labels"].reshape(-1).astype(np.int64)

    h = embed[ids]
    cw = inp["conv_w"].astype(np.float32)
    logit = h[:-1] @ cw[0, :H] + h[1:] @ cw[0, H:] + np.float32(inp["conv_b"][0])
    mask = logit > 0
    m = np.concatenate([mask, [False]])
    hn = np.where(m[:, None], 0.5 * (h + np.roll(h, -1, axis=0)), h)
    keep = np.concatenate([[True], ~mask])
    order = np.argsort(~keep, kind="stable")
    h0 = hn[order]
    lab = labels[order]
    valid_len = int(keep.sum())

    inv = 1.0 / (THETA ** (np.arange(0, HD, 2, dtype=np.float32) / HD))
    t = np.arange(S, dtype=np.float32)
    freqs = np.outer(t, inv)
    emb = np.concatenate([freqs, freqs], -1)
    cos, sin = np.cos(emb), np.sin(emb)
    sinflip = np.concatenate([-sin[:, :HD // 2], sin[:, HD // 2:]], -1)
    # rope constants absorb the 1/WS compensation for the fp8 q/k weights
    cos1 = (cos / WS).astype(bf16)
    sf1 = (sinflip / WS).astype(bf16)

    ident = np.eye(128, dtype=bf16)
    cmask = np.where(np.arange(128)[None, :] > np.arange(128)[:, None],
                     np.float32(NEG), np.float32(0)).astype(bf16)
    ones = np.ones((128, 1), dtype=bf16)

    ln1 = inp["ln1_w"].astype(np.float32)
    ln2 = inp["ln2_w"].astype(np.float32)
    normw = inp["norm_w"].astype(np.float32)
    qsc = np.float32(1.0 / np.sqrt(HD))
    lm_folded = normw[:, None] * inp["lm_head_w"].astype(np.float32)
    lm_q = _to_f8(lm_folded * WS)          # quantized once, reused for wsel
    tgt = np.concatenate([lab[1:], [0]]).astype(np.int64)
    wsel = np.ascontiguousarray(lm_q.astype(np.float32)[:, tgt] / WS).astype(bf16)

    common = dict(h0=h0.astype(bf16), cos1=cos1, sf1=sf1, ident=ident,
                  cmask=cmask, ones=ones, wsel=wsel)
    in_maps = []
    for c in range(NC_):
        mcore = dict(common)
        for l in range(L):
            qw = ln1[l][:, None] * inp["q_w"][l].astype(np.float32) * qsc * WS
            kw = ln1[l][:, None] * inp["k_w"][l].astype(np.float32) * WS
            vw = ln1[l][:, None] * inp["v_w"][l].astype(np.float32) * WS
            gw = ln2[l][:, None] * inp["gate_w"][l].astype(np.float32) * WS
            uw = ln2[l][:, None] * inp["up_w"][l].astype(np.float32) * US
            dw = inp["down_w"][l].astype(np.float32) * WS
            qkv = np.concatenate(
                [qw[:, c * 512:(c + 1) * 512],
                 kw[:, c * 128:(c + 1) * 128],
                 vw[:, c * 128:(c + 1) * 128]], 1)          # [H, 768]
            mcore[f"qkvw{l}"] = np.ascontiguousarray(
                _to_f8(qkv).reshape(32, 128, 768).transpose(1, 0, 2))
            ow = inp["o_w"][l][c * 512:(c + 1) * 512].astype(np.float32) * WS
            mcore[f"ow{l}"] = np.ascontiguousarray(
                _to_f8(ow).reshape(4, 128, H).transpose(1, 0, 2))
            gws = np.zeros((H, IP), np.float32)
            uws = np.zeros((H, IP), np.float32)
            dws = np.zeros((IP2, H), np.float32)
            gws[:, :IPC] = gw[:, c * IPC:(c + 1) * IPC]
            uws[:, :IPC] = uw[:, c * IPC:(c + 1) * IPC]
            dws[:IPC] = dw[c * IPC:(c + 1) * IPC]
            for wname, warr in ((f"gw{l}", gws), (f"uw{l}", uws)):
                out = np.zeros((3, 8, 128, 4, 512), np.float32)
                for nb in range(3):
                    NBc = 512 if nb < 2 else IP - 1024
                    blk = warr[:, nb * 512:nb * 512 + NBc]       # [H, NBc]
                    out[nb, :, :, :, :NBc] = blk.reshape(
                        8, 4, 128, NBc).transpose(0, 2, 1, 3)
                mcore[wname] = _to_f8(out)
            dout = np.zeros((8, 3, 128, 4, 512), np.float32)
            for n in range(8):
                blk = dws[:, n * 512:(n + 1) * 512]              # [IP2, 512]
                dout[n] = blk.reshape(3, 4, 128, 512).transpose(0, 2, 1, 3)
            mcore[f"dw{l}"] = _to_f8(dout)
        lmc = lm_q[:, c * VS:(c + 1) * VS].astype(np.float32)    # [H, 4000]
        lout = np.zeros((8, 8, 128, 4, 500), np.float32)
        for vb in range(8):
            blk = lmc[:, vb * 500:(vb + 1) * 500]                # [H, 500]
            lout[vb] = blk.reshape(8, 4, 128, 500).transpose(0, 2, 1, 3)
        mcore["lmw"] = _to_f8(lout)
        in_maps.append(mcore)

    return in_maps, valid_len


def kernel(**inputs) -> np.ndarray:
    in_maps, valid_len = host_prep(inputs)
    if "nc" not in _cache:
        _cache["nc"] = build_nc()
    nc = _cache["nc"]
    res = run_bass_kernel_spmd(nc, in_maps, list(range(NC_)),
                               **last_run_info.get("run_kwargs", {}))
    last_run_info["res"] = res
    out = res.results[0]
    gsum = out["gsum_o"].transpose(1, 0).reshape(S).astype(np.float64)
    tlog = out["tlog_o"].reshape(S).astype(np.float64)
    ce = LM_MAX + np.log(gsum) - tlog
    w = (np.arange(S - 1) < valid_len - 1).astype(np.float64)
    loss = (ce[:S - 1] * w).sum() / w.sum()
    return np.float32(loss)


# revision 25
# speedup vs baseline: 1.2870x; 1.0135x over previous
"""Trainium2 Bass kernel for nn_Decoder_20486994002617.  v3.

8-core tensor-parallel 2-layer llama-style decoder with ragged token-merge
(handled on host), returning the masked-mean cross-entropy loss.

v2: fp8e4 DoubleRow for qkv / o / gate / up / down / lm_head, weights
pre-scaled (x64, up-proj x4) into e4m3 range, compensation folded into rope
constants and scaled PSUM->SBUF copies; host pre-chunks weights into SBUF
tile layout so streaming DMAs are contiguous.

v3 (latency restructure):
  - AllReduces quartered (256 rows each) and posted as soon as their rows
    are ready, so consumers never wait on a half-sequence collective.
  - A persistent "bridge" array xnt[0..7] holds the normalized transposed
    activations; the residual-add + rmsnorm + transpose for each phase is
    emitted inside the *previous* phase's instruction stream (prep), so
    GEMMs start immediately at phase entry.
  - lm head uses a fixed-max (M=16) online softmax: no logits storage, no
    max AllReduce, no DRAM spill of h.
  - tlog runs in sequence halves so it can start before the last xf tiles.
"""
import numpy as np
import ml_dtypes

from contextlib import ExitStack

import concourse.bass as bass
import concourse.bacc as bacc
import concourse.mybir as mybir
import concourse.tile as tile
from concourse.bass_utils import run_bass_kernel_spmd

F32 = mybir.dt.float32
BF16 = mybir.dt.bfloat16
FP8 = mybir.dt.float8e4
AF = mybir.ActivationFunctionType
ALU = mybir.AluOpType
AX = mybir.AxisListType
DR = mybir.MatmulPerfMode.DoubleRow

H, HD, NH, NKV = 4096, 128, 32, 8
L, V, S, I = 2, 32000, 1024, 11008
EPS, THETA = 1e-6, 10000.0
NC_ = 8          # cores
IPC = I // NC_   # 1376
IP = 1408        # padded intermediate per core = 11 * 128
IP2 = 1536       # fp8-pair-padded contraction for down proj = 12 * 128
VS = V // NC_    # 4000 vocab per core
NEG = -1e9
WS = 64.0        # fp8 weight scale (qkv, o, gate, down, lm_head)
US = 4.0         # fp8 weight scale for y=silu(g)*u (y*US must stay under 240)
LM_MAX = 16.0    # fixed logsumexp shift (|logit| << 16)
SC_MAX = 12.0    # fixed softmax shift for attention scores (|score| << 12)

bf16 = ml_dtypes.bfloat16
f8 = ml_dtypes.float8_e4m3

last_run_info = {}
_cache = {}


# ----------------------------------------------------------------- device --

def _norm_transpose(nc, small, ntmp, psum, h_ap, dst, ident_sb, uid,
                    nt_tag="mix", nt_bufs=2):
    """dst[:, k, :] (32 chunks of [128,128]) = normalized transpose of
    h_ap ([128 seq rows, 4096]). dst free dims must be (32, 128)."""
    ssq = small.tile([128, 1], F32, tag="nt_ssq", bufs=2, name=f"ssq_{uid}")
    # Square scratch output goes into dst (overwritten by the transpose after)
    nc.scalar.activation(dst, h_ap.rearrange("p (k m) -> p k m", k=32),
                         AF.Square, accum_out=ssq[:])
    var = small.tile([128, 1], F32, tag="nt_var", bufs=2, name=f"var_{uid}")
    nc.vector.tensor_scalar(var[:], ssq[:], 1.0 / H, EPS, op0=ALU.mult, op1=ALU.add)
    std = small.tile([128, 1], F32, tag="nt_std", bufs=2, name=f"std_{uid}")
    nc.scalar.sqrt(std[:], var[:])
    fac = small.tile([128, 1], F32, tag="nt_fac", bufs=2, name=f"fac_{uid}")
    nc.vector.reciprocal(fac[:], std[:])
    diag = ntmp.tile([128, 128], BF16, tag="nt_diag", bufs=2, name=f"diag_{uid}")
    nc.vector.tensor_scalar_mul(diag[:], ident_sb[:], fac[:])
    for kk in range(8):
        pnt = psum.tile([128, 512], F32, tag=nt_tag, bufs=nt_bufs,
                        name=f"pnt_{uid}_{kk}")
        for j in range(4):
            k = kk * 4 + j
            nc.tensor.matmul(pnt[:, j * 128:(j + 1) * 128],
                             h_ap[:, k * 128:(k + 1) * 128], diag[:],
                             start=True, stop=True)
        nc.any.tensor_copy(dst[:, kk * 4:(kk + 1) * 4, :],
                           pnt[:].rearrange("p (j m) -> p j m", j=4))


def _rope(nc, ntmp, ps, cos_ap, sf_ap, out, nheads, i):
    """out (bf16 [128, nheads*128]) = rope(ps); cos_ap/sf_ap are [128,128]."""
    n = nheads * 128
    t1 = ntmp.tile([128, 512], F32, tag="rope_t1", bufs=1, name=f"t1_{i}_{nheads}")
    t2 = ntmp.tile([128, 512], F32, tag="rope_t2", bufs=1, name=f"t2_{i}_{nheads}")
    for hh in range(nheads):
        b = hh * 128
        nc.vector.tensor_mul(t1[:, b:b + 128], ps[:, b:b + 128], cos_ap)
        nc.vector.tensor_mul(t2[:, b:b + 64], ps[:, b + 64:b + 128],
                             sf_ap[:, 0:64])
        nc.vector.tensor_mul(t2[:, b + 64:b + 128], ps[:, b:b + 64],
                             sf_ap[:, 64:128])
    nc.vector.tensor_add(out[:], t1[:, :n], t2[:, :n])


def build_nc():
    nc = bacc.Bacc("TRN2", target_bir_lowering=False, debug=False,
                   num_devices=NC_)

    din = {}
    def dram_in(name, shape, dtype=BF16):
        din[name] = nc.dram_tensor(name, shape, dtype, kind="ExternalInput")
        return din[name]

    h0_d = dram_in("h0", [S, H])
    cos1_d = dram_in("cos1", [S, 128])
    sf1_d = dram_in("sf1", [S, 128])
    ident_d = dram_in("ident", [128, 128])
    cmask_d = dram_in("cmask", [128, 128])
    ones_d = dram_in("ones", [128, 1])
    for l in range(L):
        dram_in(f"qkvw{l}", [128, 32, 768], FP8)
        dram_in(f"ow{l}", [128, 4, H], FP8)
        dram_in(f"gw{l}", [3, 8, 128, 4, 512], FP8)   # [nb, kp, p, j, n]
        dram_in(f"uw{l}", [3, 8, 128, 4, 512], FP8)
        dram_in(f"dw{l}", [8, 3, 128, 4, 512], FP8)   # [n, tp, p, j, n]
    lmw_d = dram_in("lmw", [8, 8, 128, 4, 500], FP8)  # [vb, kp, p, j, n]
    wsel_d = dram_in("wsel", [H, S])

    gsum_o = nc.dram_tensor("gsum_o", [128, 8], F32, kind="ExternalOutput")
    tlog_o = nc.dram_tensor("tlog_o", [1, S], F32, kind="ExternalOutput")

    rg = [list(range(NC_))]

    with tile.TileContext(nc) as tc:
        with (
            tc.tile_pool(name="pconst", bufs=1) as pconst,
            tc.tile_pool(name="psmall", bufs=1) as psmall,
            tc.tile_pool(name="pbridge", bufs=1) as pbridge,
            tc.tile_pool(name="pdram", bufs=1, space="DRAM") as pdram,
        ):
            ident_sb = pconst.tile([128, 128], BF16)
            cmask_sb = pconst.tile([128, 128], BF16)
            ones_sb = pconst.tile([128, 1], BF16)
            cos_sb = pconst.tile([128, 8, 128], BF16)
            sf_sb = pconst.tile([128, 8, 128], BF16)
            negSM = pconst.tile([128, 1], F32)
            nc.any.memset(negSM[:], -SC_MAX)
            nc.sync.dma_start(ident_sb[:], ident_d.ap())
            nc.sync.dma_start(cmask_sb[:], cmask_d.ap())
            nc.sync.dma_start(ones_sb[:], ones_d.ap())
            for i in range(8):
                nc.sync.dma_start(cos_sb[:, i, :], cos1_d.ap()[i * 128:(i + 1) * 128, :])
                nc.sync.dma_start(sf_sb[:, i, :], sf1_d.ap()[i * 128:(i + 1) * 128, :])

            xnt = [pbridge.tile([128, 32, 128], FP8, name=f"xnt_{j}")
                   for j in range(8)]

            hstack = ExitStack()
            phh = hstack.enter_context(tc.tile_pool(name="phh", bufs=1))
            h_sb = phh.tile([128, 8, H], BF16)
            for i in range(8):
                nc.sync.dma_start(h_sb[:, i, :], h0_d.ap()[i * 128:(i + 1) * 128, :])

            # quarter-grained AR buffers: [4 quarters][256, H]
            ar_ins, ar_outss, ar2_ins, ar2_outss = [], [], [], []
            for l in range(L):
                ar_ins.append(pdram.tile([S, H], BF16, name=f"ar_in_{l}"))
                ar_outss.append([pdram.tile([256, H], BF16, addr_space="Shared",
                                            name=f"ar_out_{l}_{q}")
                                 for q in range(4)])
                ar2_ins.append(pdram.tile([S, H], BF16, name=f"ar2_in_{l}"))
                ar2_outss.append([pdram.tile([256, H], BF16, addr_space="Shared",
                                             name=f"ar2_out_{l}_{q}")
                                  for q in range(4)])

            def prep(pool, psum, j, res_q, dst, uid, nt_tag="mix", nt_bufs=2):
                """h_sb[:,j] += AR-quarter residual; dst = norm-transpose."""
                if res_q is not None:
                    rt = pool.tile([128, H], BF16, tag="prep_rt", bufs=2,
                                   name=f"rt_{uid}")
                    nc.sync.dma_start(
                        rt[:], res_q[(j % 2) * 128:(j % 2 + 1) * 128, :])
                    nc.vector.tensor_add(h_sb[:, j, :], h_sb[:, j, :], rt[:])
                _norm_transpose(nc, psmall, pool, psum, h_sb[:, j, :], dst,
                                ident_sb, uid, nt_tag=nt_tag, nt_bufs=nt_bufs)

            xfstack = ExitStack()

            for l in range(L):
                # ======== attention: per-tile qkv -> heads -> o-proj ========
                with (
                    tc.tile_pool(name="pal", bufs=1) as pal,
                    tc.tile_pool(name="paps", bufs=1, space="PSUM") as paps,
                ):
                    kT_sb = pal.tile([128, S], BF16)
                    v_sb = pal.tile([128, 8, 128], BF16)
                    ar_in = ar_ins[l]
                    ar_outs = ar_outss[l]
                    wqkv_sb = pal.tile([128, 32, 768], FP8)
                    ow_sb = pal.tile([128, 4, H], FP8)
                    nc.sync.dma_start(wqkv_sb[:], din[f"qkvw{l}"].ap())
                    nc.sync.dma_start(ow_sb[:], din[f"ow{l}"].ap())
                    if l == 0:
                        for j in range(8):
                            prep(pal, paps, j, None, xnt[j], f"i{j}")
                    for i in range(8):
                        if l > 0 and i in (0, 1, 3, 4):
                            # prep the second-half attention tiles of this
                            # layer as their ar2 quarters land (q3 is late)
                            j = {0: 4, 1: 5, 3: 6, 4: 7}[i]
                            prep(pal, paps, j, ar2_outss[l - 1][j // 2],
                                 xnt[j], f"a{l}_{j}")
                        psq = paps.tile([128, 512], F32, tag="psq", bufs=1,
                                        name=f"psq_{l}_{i}")
                        pskv = paps.tile([128, 256], F32, tag="pskv", bufs=1,
                                         name=f"pskv_{l}_{i}")
                        for k in range(16):
                            nc.tensor.matmul(psq[:], xnt[i][:, 2 * k:2 * k + 2, :],
                                             wqkv_sb[:, 2 * k:2 * k + 2, 0:512],
                                             start=(k == 0), stop=(k == 15),
                                             perf_mode=DR)
                            nc.tensor.matmul(pskv[:], xnt[i][:, 2 * k:2 * k + 2, :],
                                             wqkv_sb[:, 2 * k:2 * k + 2, 512:768],
                                             start=(k == 0), stop=(k == 15),
                                             perf_mode=DR)
                        qT_sb = pal.tile([128, 4, 128], BF16, tag="qT",
                                         bufs=2, name=f"qT_{l}_{i}")
                        oT_sb = pal.tile([128, 4, 128], FP8, tag="oT",
                                         bufs=2, name=f"oT_{l}_{i}")
                        q_rot = pal.tile([128, 512], BF16, tag="q_rot", bufs=2,
                                         name=f"qr_{l}_{i}")
                        k_rot = pal.tile([128, 128], BF16, tag="k_rot", bufs=2,
                                         name=f"kr_{l}_{i}")
                        _rope(nc, pal, psq[:], cos_sb[:, i, :], sf_sb[:, i, :],
                              q_rot, 4, f"{l}_{i}")
                        _rope(nc, pal, pskv[:, 0:128], cos_sb[:, i, :],
                              sf_sb[:, i, :], k_rot, 1, f"{l}_{i}")
                        nc.vector.tensor_scalar_mul(v_sb[:, i, :],
                                                    pskv[:, 128:256], 1.0 / WS)
                        ptr = paps.tile([128, 512], F32, tag="mix", bufs=2,
                                        name=f"ptrq_{l}_{i}")
                        for hh in range(4):
                            nc.tensor.matmul(ptr[:, hh * 128:(hh + 1) * 128],
                                             q_rot[:, hh * 128:(hh + 1) * 128],
                                             ident_sb[:], start=True, stop=True)
                        nc.any.tensor_copy(qT_sb[:],
                                           ptr[:].rearrange("p (j m) -> p j m", j=4))
                        ptrk = paps.tile([128, 512], F32, tag="mix", bufs=2,
                                         name=f"ptrk_{l}_{i}")
                        nc.tensor.matmul(ptrk[:, :128], k_rot[:], ident_sb[:],
                                         start=True, stop=True)
                        nc.any.tensor_copy(kT_sb[:, i * 128:(i + 1) * 128], ptrk[:, :128])
                        n2 = 128 * (i + 1)
                        for hh in range(4):
                            pss = paps.tile([128, 1024], F32, tag="pss", bufs=1,
                                            name=f"pss_{l}_{hh}_{i}")
                            lhs_q = qT_sb[:, hh, :]
                            c0 = 0
                            while c0 < n2 - 128:
                                N = min(512, n2 - 128 - c0)
                                nc.tensor.matmul(pss[:, c0:c0 + N], lhs_q,
                                                 kT_sb[:, c0:c0 + N],
                                                 start=True, stop=True)
                                c0 += N
                            nc.tensor.matmul(pss[:, n2 - 128:n2], lhs_q,
                                             kT_sb[:, n2 - 128:n2],
                                             start=True, stop=False)
                            nc.tensor.matmul(pss[:, n2 - 128:n2], ident_sb[:],
                                             cmask_sb[:], start=False, stop=True)
                            sume = psmall.tile([128, 1], F32, tag="sume", bufs=2,
                                               name=f"sume_{l}_{hh}_{i}")
                            exp_sb = pal.tile([128, 1024], BF16, tag="exp", bufs=2,
                                              name=f"exp_{l}_{hh}_{i}")
                            nc.scalar.activation(exp_sb[:, :n2], pss[:, :n2], AF.Exp,
                                                 bias=negSM[:], accum_out=sume[:])
                            rec = psmall.tile([128, 1], F32, tag="rec", bufs=2,
                                              name=f"rec_{l}_{hh}_{i}")
                            nc.vector.reciprocal(rec[:], sume[:])
                            diag_r = pal.tile([128, 128], BF16, tag="diag_r", bufs=2,
                                              name=f"diagr_{l}_{hh}_{i}")
                            nc.vector.tensor_scalar_mul(diag_r[:], ident_sb[:], rec[:])
                            atcol = pal.tile([128, 8, 128], BF16, tag="atcol", bufs=2,
                                             name=f"atcol_{l}_{hh}_{i}")
                            for jg in range(0, i + 1, 4):
                                ng = min(4, i + 1 - jg)
                                pat = paps.tile([128, 512], F32, tag="mix", bufs=2,
                                                name=f"pat_{l}_{hh}_{i}_{jg}")
                                for jj in range(ng):
                                    j = jg + jj
                                    nc.tensor.matmul(pat[:, jj * 128:(jj + 1) * 128],
                                                     exp_sb[:, j * 128:(j + 1) * 128],
                                                     diag_r[:], start=True, stop=True)
                                nc.any.tensor_copy(
                                    atcol[:, jg:jg + ng, :],
                                    pat[:, :ng * 128].rearrange("p (j m) -> p j m", j=ng))
                            pso = paps.tile([128, 128], F32, tag="pso", bufs=2,
                                            name=f"pso_{l}_{hh}_{i}")
                            for j in range(i + 1):
                                nc.tensor.matmul(pso[:], v_sb[:, j, :], atcol[:, j, :],
                                                 start=(j == 0), stop=(j == i))
                            nc.any.tensor_copy(oT_sb[:, hh, :], pso[:])
                        ob = pal.tile([128, H], BF16, tag="ob", bufs=1,
                                      name=f"ob_{l}_{i}")
                        for n in range(8):
                            pps = paps.tile([128, 512], F32, tag="mix", bufs=2,
                                            name=f"pop_{l}_{i}_{n}")
                            for t in range(2):
                                nc.tensor.matmul(pps[:], oT_sb[:, 2 * t:2 * t + 2, :],
                                                 ow_sb[:, 2 * t:2 * t + 2,
                                                       n * 512:(n + 1) * 512],
                                                 start=(t == 0), stop=(t == 1),
                                                 perf_mode=DR)
                            nc.vector.tensor_scalar_mul(ob[:, n * 512:(n + 1) * 512],
                                                        pps[:], 1.0 / WS)
                        nc.sync.dma_start(ar_in[i * 128:(i + 1) * 128, :], ob[:])
                        if i % 2 == 1:
                            q = i // 2
                            nc.gpsimd.collective_compute(
                                "AllReduce", ALU.add, replica_groups=rg,
                                ins=[ar_in[q * 256:(q + 1) * 256, :].opt()],
                                outs=[ar_outs[q].opt()])
                        if i >= 4:
                            # prep MLP tiles 0..3 (attn AR quarters 0,1 ready)
                            j = i - 4
                            prep(pal, paps, j, ar_outs[j // 2], xnt[j],
                                 f"m{l}_{j}")
                    # prep MLP tiles 4,5 (quarter 2 posted after i==5)
                    for j in (4, 5):
                        prep(pal, paps, j, ar_outs[j // 2], xnt[j], f"m{l}_{j}")

                # ===== MLP: gate/up -> down in row-quarters -> AR2 ==========
                if l == L - 1:
                    pxf = xfstack.enter_context(tc.tile_pool(name="pxf", bufs=1))
                    xf_sb = pxf.tile([128, 32, S], FP8)
                with (
                    tc.tile_pool(name="pml", bufs=1) as pml,
                    tc.tile_pool(name="pmps", bufs=1, space="PSUM") as pmps,
                ):
                    ar2_in = ar2_ins[l]
                    ar2_outs = ar2_outss[l]

                    def prep_next(j):
                        """prep for the next phase: attention l+1 (into xnt)
                        or the final norm (into xf)."""
                        if l < L - 1:
                            prep(pml, pmps, j, ar2_outs[j // 2], xnt[j],
                                 f"a{l + 1}_{j}", nt_tag="mlpps", nt_bufs=4)
                        else:
                            prep(pml, pmps, j, ar2_outs[j // 2],
                                 xf_sb[:, :, j * 128:(j + 1) * 128],
                                 f"f{j}", nt_tag="mlpps", nt_bufs=4)

                    for ig in range(2):
                        with tc.tile_pool(name="pgu", bufs=1) as pgu:
                            if ig == 1:
                                # prep MLP tiles 6,7 (attn AR q3 long done)
                                for j in (6, 7):
                                    prep(pml, pmps, j, ar_outs[j // 2], xnt[j],
                                         f"m{l}_{j}", nt_tag="mlpps", nt_bufs=4)
                            yt_sb = pml.tile([128, 12, 512], FP8, tag="yt",
                                             bufs=2, name=f"yt_{l}_{ig}")
                            nc.vector.memset(yt_sb[:, 11, :], 0.0)
                            gu = {}
                            for wi, (wname, tag) in enumerate(
                                    ((f"gw{l}", "g"), (f"uw{l}", "u"))):
                                outs = [pgu.tile([128, IP], BF16, tag=tag, bufs=4,
                                                 name=f"{tag}_{l}_{ig}_{ii}")
                                        for ii in range(4)]
                                gu[tag] = outs
                                for nb in range(3):
                                    NB = 512 if nb < 2 else IP - 1024
                                    pg = [pmps.tile([128, 512], F32, tag="mlpps", bufs=4,
                                                    name=f"pg_{l}_{ig}_{tag}_{nb}_{ii}")
                                          for ii in range(4)]
                                    for kp in range(8):
                                        wt = pgu.tile([128, 4, 512], FP8, tag="wstream",
                                                      bufs=4,
                                                      name=f"wt_{l}_{ig}_{tag}_{nb}_{kp}")
                                        nc.sync.dma_start(wt[:], din[wname].ap()[nb, kp])
                                        for jp in range(2):
                                            k = kp * 4 + 2 * jp
                                            for ii in range(4):
                                                nc.tensor.matmul(
                                                    pg[ii][:, :NB],
                                                    xnt[ig * 4 + ii][:, k:k + 2, :],
                                                    wt[:, 2 * jp:2 * jp + 2, :NB],
                                                    start=(k == 0), stop=(k == 30),
                                                    perf_mode=DR)
                                    for ii in range(4):
                                        nc.any.tensor_copy(
                                            outs[ii][:, nb * 512:nb * 512 + NB],
                                            pg[ii][:, :NB])
                                    # interleave next-phase preps into ig=1
                                    if ig == 1 and wi == 0:
                                        prep_next(nb)          # tiles 0,1,2
                                    if ig == 1 and wi == 1 and nb == 0:
                                        prep_next(3)
                            for iq in range(2):
                                for ii in (iq * 2, iq * 2 + 1):
                                    i = ig * 4 + ii
                                    ysil = pgu.tile([128, IP], BF16, tag="ysil", bufs=2,
                                                    name=f"ysil_{l}_{i}")
                                    nc.scalar.activation(ysil[:], gu["g"][ii][:], AF.Silu,
                                                         scale=1.0 / WS)
                                    y = gu["u"][ii]
                                    nc.vector.tensor_mul(y[:], ysil[:], y[:])
                                    for tq in range(3):
                                        ts = [tq * 4 + j for j in range(4) if tq * 4 + j < 11]
                                        ptr = pmps.tile([128, 512], F32, tag="mlpps", bufs=4,
                                                        name=f"ytr_{l}_{i}_{tq}")
                                        for jj, t in enumerate(ts):
                                            nc.tensor.matmul(ptr[:, jj * 128:(jj + 1) * 128],
                                                             y[:, t * 128:(t + 1) * 128],
                                                             ident_sb[:], start=True, stop=True)
                                        nc.any.tensor_copy(
                                            yt_sb[:, ts[0]:ts[0] + len(ts),
                                                  ii * 128:(ii + 1) * 128],
                                            ptr[:, :len(ts) * 128].rearrange(
                                                "p (j m) -> p j m", j=len(ts)))
                                for n in range(8):
                                    pd = [pmps.tile([128, 512], F32, tag=f"pd{i2}", bufs=1,
                                                    name=f"pd_{l}_{ig}_{iq}_{n}_{i2}")
                                          for i2 in range(2)]
                                    for tp in range(3):
                                        dwt = pgu.tile([128, 4, 512], FP8, tag="dwstream",
                                                       bufs=4,
                                                       name=f"dwt_{l}_{ig}_{iq}_{n}_{tp}")
                                        nc.sync.dma_start(dwt[:], din[f"dw{l}"].ap()[n, tp])
                                        for jp in range(2):
                                            c = tp * 4 + 2 * jp
                                            for i2 in range(2):
                                                ii = iq * 2 + i2
                                                nc.tensor.matmul(
                                                    pd[i2][:],
                                                    yt_sb[:, c:c + 2,
                                                          ii * 128:(ii + 1) * 128],
                                                    dwt[:, 2 * jp:2 * jp + 2, :],
                                                    start=(c == 0), stop=(c == 10),
                                                    perf_mode=DR)
                                    for i2 in range(2):
                                        i = ig * 4 + iq * 2 + i2
                                        db = pgu.tile([128, 512], BF16, tag="db", bufs=2,
                                                      name=f"db_{l}_{ig}_{iq}_{n}_{i2}")
                                        nc.vector.tensor_scalar_mul(db[:], pd[i2][:],
                                                                    1.0 / (WS * US))
                                        nc.sync.dma_start(
                                            ar2_in[i * 128:(i + 1) * 128,
                                                   n * 512:(n + 1) * 512], db[:])
                                q = ig * 2 + iq
                                nc.gpsimd.collective_compute(
                                    "AllReduce", ALU.add, replica_groups=rg,
                                    ins=[ar2_in[q * 256:(q + 1) * 256, :].opt()],
                                    outs=[ar2_outs[q].opt()])
                    if l == L - 1:
                        # final xf tiles 4..7 (ar2 q2/q3 land late) with the
                        # tlog halves interleaved to hide the tail AR latency
                        with (
                            tc.tile_pool(name="ptl", bufs=1) as ptl,
                            tc.tile_pool(name="ptps", bufs=1, space="PSUM") as ptps,
                        ):
                            pts = [ptps.tile([1, 512], F32, name=f"pt{h_}")
                                   for h_ in range(2)]

                            def tlog_half(half):
                                for kp in range(8):
                                    ws = ptl.tile([128, 4, 512], BF16, tag="wsel",
                                                  bufs=2, name=f"ws_{half}_{kp}")
                                    nc.sync.dma_start(
                                        ws[:], wsel_d.ap()[kp * 512:(kp + 1) * 512,
                                                           half * 512:(half + 1) * 512]
                                        .rearrange("(j p) n -> p j n", p=128))
                                    for jk in range(4):
                                        k = kp * 4 + jk
                                        tm = ptl.tile([128, 512], BF16, tag="tm",
                                                      bufs=2, name=f"tm_{half}_{k}")
                                        nc.vector.tensor_mul(
                                            tm[:], xf_sb[:, k,
                                                         half * 512:(half + 1) * 512],
                                            ws[:, jk, :])
                                        nc.tensor.matmul(pts[half][:], ones_sb[:],
                                                         tm[:], start=(k == 0),
                                                         stop=(k == 31))

                            prep_next(4)
                            prep_next(5)
                            tlog_half(0)
                            prep_next(6)
                            prep_next(7)
                            tlog_half(1)
                            tl_sb = ptl.tile([1, S], F32)
                            nc.any.tensor_copy(tl_sb[:, :512], pts[0][:])
                            nc.any.tensor_copy(tl_sb[:, 512:], pts[1][:])
                            nc.sync.dma_start(tlog_o.ap(), tl_sb[:])

            # ==================== lm (online fixed-max softmax) =============

            with (
                tc.tile_pool(name="plm", bufs=1) as plm,
                tc.tile_pool(name="plps", bufs=1, space="PSUM") as plps,
                tc.tile_pool(name="pld", bufs=1, space="DRAM") as pld,
            ):
                s_sb = plm.tile([128, 8], F32)
                nc.any.memset(s_sb[:], 0.0)
                negM = plm.tile([128, 1], F32)
                nc.any.memset(negM[:], -LM_MAX)
                for vb in range(8):
                    pl = [plps.tile([128, 500], F32, tag=f"pl{i}", bufs=1,
                                    name=f"pl_{vb}_{i}") for i in range(8)]
                    for kp in range(8):
                        lt = plm.tile([128, 4, 500], FP8, tag="lmw", bufs=4,
                                      name=f"lt_{vb}_{kp}")
                        nc.sync.dma_start(lt[:], lmw_d.ap()[vb, kp])
                        for jp in range(2):
                            k = kp * 4 + 2 * jp
                            for i in range(8):
                                nc.tensor.matmul(pl[i][:],
                                                 xf_sb[:, k:k + 2, i * 128:(i + 1) * 128],
                                                 lt[:, 2 * jp:2 * jp + 2, :],
                                                 start=(k == 0), stop=(k == 30),
                                                 perf_mode=DR)
                    for i in range(8):
                        se = psmall.tile([128, 1], F32, tag="se", bufs=2,
                                         name=f"se_{vb}_{i}")
                        scr = plm.tile([128, 500], BF16, tag="scr", bufs=2,
                                       name=f"scr_{vb}_{i}")
                        nc.scalar.activation(scr[:], pl[i][:], AF.Exp,
                                             bias=negM[:], scale=1.0 / WS,
                                             accum_out=se[:])
                        nc.vector.tensor_add(s_sb[:, i:i + 1], s_sb[:, i:i + 1],
                                             se[:])
                gs_in = pld.tile([128, 8], F32)
                gs_out = pld.tile([128, 8], F32, addr_space="Shared")
                nc.sync.dma_start(gs_in[:], s_sb[:])
                nc.gpsimd.collective_compute("AllReduce", ALU.add, replica_groups=rg,
                                             ins=[gs_in.opt()], outs=[gs_out.opt()])
                gsf_sb = plm.tile([128, 8], F32)
                nc.sync.dma_start(gsf_sb[:], gs_out[:])
                nc.sync.dma_start(gsum_o.ap(), gsf_sb[:])
            xfstack.close()
            hstack.close()

    nc.compile()
    return nc


# ------------------------------------------------------------------- host --

def _to_f8(x):
    return np.clip(x, -240.0, 240.0).astype(f8)


def host_prep(inputs):
    inp = {k: np.asarray(v) for k, v in inputs.items()}
    embed = inp["embed"].astype(np.float32)
    ids = inp["input_ids"].reshape(-1).astype(np.int64)
    labels = inp["labels"].reshape(-1).astype(np.int64)

    h = embed[ids]
    cw = inp["conv_w"].astype(np.float32)
    logit = h[:-1] @ cw[0, :H] + h[1:] @ cw[0, H:] + np.float32(inp["conv_b"][0])
    mask = logit > 0
    m = np.concatenate([mask, [False]])
    hn = np.where(m[:, None], 0.5 * (h + np.roll(h, -1, axis=0)), h)
    keep = np.concatenate([[True], ~mask])
    order = np.argsort(~keep, kind="stable")
    h0 = hn[order]
    lab = labels[order]
    valid_len = int(keep.sum())

    inv = 1.0 / (THETA ** (np.arange(0, HD, 2, dtype=np.float32) / HD))
    t = np.arange(S, dtype=np.float32)
    freqs = np.outer(t, inv)
    emb = np.concatenate([freqs, freqs], -1)
    cos, sin = np.cos(emb), np.sin(emb)
    sinflip = np.concatenate([-sin[:, :HD // 2], sin[:, HD // 2:]], -1)
    # rope constants absorb the 1/WS compensation for the fp8 q/k weights
    cos1 = (cos / WS).astype(bf16)
    sf1 = (sinflip / WS).astype(bf16)

    ident = np.eye(128, dtype=bf16)
    cmask = np.where(np.arange(128)[None, :] > np.arange(128)[:, None],
                     np.float32(NEG), np.float32(0)).astype(bf16)
    ones = np.ones((128, 1), dtype=bf16)

    ln1 = inp["ln1_w"].astype(np.float32)
    ln2 = inp["ln2_w"].astype(np.float32)
    normw = inp["norm_w"].astype(np.float32)
    qsc = np.float32(1.0 / np.sqrt(HD))
    lm_folded = normw[:, None] * inp["lm_head_w"].astype(np.float32)
    lm_q = _to_f8(lm_folded * WS)          # quantized once, reused for wsel
    tgt = np.concatenate([lab[1:], [0]]).astype(np.int64)
    wsel = np.ascontiguousarray(lm_q.astype(np.float32)[:, tgt] / WS).astype(bf16)

    common = dict(h0=h0.astype(bf16), cos1=cos1, sf1=sf1, ident=ident,
                  cmask=cmask, ones=ones, wsel=wsel)
    in_maps = []
    for c in range(NC_):
        mcore = dict(common)
        for l in range(L):
            qw = ln1[l][:, None] * inp["q_w"][l].astype(np.float32) * qsc * WS
            kw = ln1[l][:, None] * inp["k_w"][l].astype(np.float32) * WS
            vw = ln1[l][:, None] * inp["v_w"][l].astype(np.float32) * WS
            gw = ln2[l][:, None] * inp["gate_w"][l].astype(np.float32) * WS
            uw = ln2[l][:, None] * inp["up_w"][l].astype(np.float32) * US
            dw = inp["down_w"][l].astype(np.float32) * WS
            qkv = np.concatenate(
                [qw[:, c * 512:(c + 1) * 512],
                 kw[:, c * 128:(c + 1) * 128],
                 vw[:, c * 128:(c + 1) * 128]], 1)          # [H, 768]
            mcore[f"qkvw{l}"] = np.ascontiguousarray(
                _to_f8(qkv).reshape(32, 128, 768).transpose(1, 0, 2))
            ow = inp["o_w"][l][c * 512:(c + 1) * 512].astype(np.float32) * WS
            mcore[f"ow{l}"] = np.ascontiguousarray(
                _to_f8(ow).reshape(4, 128, H).transpose(1, 0, 2))
            gws = np.zeros((H, IP), np.float32)
            uws = np.zeros((H, IP), np.float32)
            dws = np.zeros((IP2, H), np.float32)
            gws[:, :IPC] = gw[:, c * IPC:(c + 1) * IPC]
            uws[:, :IPC] = uw[:, c * IPC:(c + 1) * IPC]
            dws[:IPC] = dw[c * IPC:(c + 1) * IPC]
            for wname, warr in ((f"gw{l}", gws), (f"uw{l}", uws)):
                out = np.zeros((3, 8, 128, 4, 512), np.float32)
                for nb in range(3):
                    NBc = 512 if nb < 2 else IP - 1024
                    blk = warr[:, nb * 512:nb * 512 + NBc]       # [H, NBc]
                    out[nb, :, :, :, :NBc] = blk.reshape(
                        8, 4, 128, NBc).transpose(0, 2, 1, 3)
                mcore[wname] = _to_f8(out)
            dout = np.zeros((8, 3, 128, 4, 512), np.float32)
            for n in range(8):
                blk = dws[:, n * 512:(n + 1) * 512]              # [IP2, 512]
                dout[n] = blk.reshape(3, 4, 128, 512).transpose(0, 2, 1, 3)
            mcore[f"dw{l}"] = _to_f8(dout)
        lmc = lm_q[:, c * VS:(c + 1) * VS].astype(np.float32)    # [H, 4000]
        lout = np.zeros((8, 8, 128, 4, 500), np.float32)
        for vb in range(8):
            blk = lmc[:, vb * 500:(vb + 1) * 500]                # [H, 500]
            lout[vb] = blk.reshape(8, 4, 128, 500).transpose(0, 2, 1, 3)
        mcore["lmw"] = _to_f8(lout)
        in_maps.append(mcore)

    return in_maps, valid_len


def kernel(**inputs) -> np.ndarray:
    in_maps, valid_len = host_prep(inputs)
    if "nc" not in _cache:
        _cache["nc"] = build_nc()
    nc = _cache["nc"]
    res = run_bass_kernel_spmd(nc, in_maps, list(range(NC_)),
                               **last_run_info.get("run_kwargs", {}))
    last_run_info["res"] = res
    out = res.results[0]
    gsum = out["gsum_o"].transpose(1, 0).reshape(S).astype(np.float64)
    tlog = out["tlog_o"].reshape(S).astype(np.float64)
    ce = LM_MAX + np.log(gsum) - tlog
    w = (np.arange(S - 1) < valid_len - 1).astype(np.float64)
    loss = (ce[:S - 1] * w).sum() / w.sum()
    return np.float32(loss)


# revision 26
# speedup vs baseline: 1.2981x; 1.0086x over previous
"""Trainium2 Bass kernel for nn_Decoder_20486994002617.  v3.

8-core tensor-parallel 2-layer llama-style decoder with ragged token-merge
(handled on host), returning the masked-mean cross-entropy loss.

v2: fp8e4 DoubleRow for qkv / o / gate / up / down / lm_head, weights
pre-scaled (x64, up-proj x4) into e4m3 range, compensation folded into rope
constants and scaled PSUM->SBUF copies; host pre-chunks weights into SBUF
tile layout so streaming DMAs are contiguous.

v3 (latency restructure):
  - AllReduces quartered (256 rows each) and posted as soon as their rows
    are ready, so consumers never wait on a half-sequence collective.
  - A persistent "bridge" array xnt[0..7] holds the normalized transposed
    activations; the residual-add + rmsnorm + transpose for each phase is
    emitted inside the *previous* phase's instruction stream (prep), so
    GEMMs start immediately at phase entry.
  - lm head uses a fixed-max (M=16) online softmax: no logits storage, no
    max AllReduce, no DRAM spill of h.
  - tlog runs in sequence halves so it can start before the last xf tiles.
"""
import numpy as np
import ml_dtypes

from contextlib import ExitStack

import concourse.bass as bass
import concourse.bacc as bacc
import concourse.mybir as mybir
import concourse.tile as tile
from concourse.bass_utils import run_bass_kernel_spmd

F32 = mybir.dt.float32
BF16 = mybir.dt.bfloat16
FP8 = mybir.dt.float8e4
AF = mybir.ActivationFunctionType
ALU = mybir.AluOpType
AX = mybir.AxisListType
DR = mybir.MatmulPerfMode.DoubleRow

H, HD, NH, NKV = 4096, 128, 32, 8
L, V, S, I = 2, 32000, 1024, 11008
EPS, THETA = 1e-6, 10000.0
NC_ = 8          # cores
IPC = I // NC_   # 1376
IP = 1408        # padded intermediate per core = 11 * 128
IP2 = 1536       # fp8-pair-padded contraction for down proj = 12 * 128
VS = V // NC_    # 4000 vocab per core
NEG = -1e9
WS = 64.0        # fp8 weight scale (qkv, o, gate, down, lm_head)
US = 4.0         # fp8 weight scale for y=silu(g)*u (y*US must stay under 240)
LM_MAX = 16.0    # fixed logsumexp shift (|logit| << 16)
SC_MAX = 12.0    # fixed softmax shift for attention scores (|score| << 12)

bf16 = ml_dtypes.bfloat16
f8 = ml_dtypes.float8_e4m3

last_run_info = {}
_cache = {}


# ----------------------------------------------------------------- device --

def _norm_transpose(nc, small, ntmp, psum, h_ap, dst, ident_sb, uid,
                    nt_tag="mix", nt_bufs=2):
    """dst[:, k, :] (32 chunks of [128,128]) = normalized transpose of
    h_ap ([128 seq rows, 4096]). dst free dims must be (32, 128)."""
    ssq = small.tile([128, 1], F32, tag="nt_ssq", bufs=2, name=f"ssq_{uid}")
    # Square scratch output goes into dst (overwritten by the transpose after)
    nc.scalar.activation(dst, h_ap.rearrange("p (k m) -> p k m", k=32),
                         AF.Square, accum_out=ssq[:])
    var = small.tile([128, 1], F32, tag="nt_var", bufs=2, name=f"var_{uid}")
    nc.vector.tensor_scalar(var[:], ssq[:], 1.0 / H, EPS, op0=ALU.mult, op1=ALU.add)
    std = small.tile([128, 1], F32, tag="nt_std", bufs=2, name=f"std_{uid}")
    nc.scalar.sqrt(std[:], var[:])
    fac = small.tile([128, 1], F32, tag="nt_fac", bufs=2, name=f"fac_{uid}")
    nc.vector.reciprocal(fac[:], std[:])
    diag = ntmp.tile([128, 128], BF16, tag="nt_diag", bufs=2, name=f"diag_{uid}")
    nc.vector.tensor_scalar_mul(diag[:], ident_sb[:], fac[:])
    for kk in range(8):
        pnt = psum.tile([128, 512], F32, tag=nt_tag, bufs=nt_bufs,
                        name=f"pnt_{uid}_{kk}")
        for j in range(4):
            k = kk * 4 + j
            nc.tensor.matmul(pnt[:, j * 128:(j + 1) * 128],
                             h_ap[:, k * 128:(k + 1) * 128], diag[:],
                             start=True, stop=True)
        nc.any.tensor_copy(dst[:, kk * 4:(kk + 1) * 4, :],
                           pnt[:].rearrange("p (j m) -> p j m", j=4))


def _rope(nc, ntmp, ps, cos_ap, sf_ap, out, nheads, i):
    """out (bf16 [128, nheads*128]) = rope(ps); cos_ap/sf_ap are [128,128]."""
    n = nheads * 128
    t1 = ntmp.tile([128, 512], F32, tag="rope_t1", bufs=1, name=f"t1_{i}_{nheads}")
    t2 = ntmp.tile([128, 512], F32, tag="rope_t2", bufs=1, name=f"t2_{i}_{nheads}")
    for hh in range(nheads):
        b = hh * 128
        nc.vector.tensor_mul(t1[:, b:b + 128], ps[:, b:b + 128], cos_ap)
        nc.vector.tensor_mul(t2[:, b:b + 64], ps[:, b + 64:b + 128],
                             sf_ap[:, 0:64])
        nc.vector.tensor_mul(t2[:, b + 64:b + 128], ps[:, b:b + 64],
                             sf_ap[:, 64:128])
    nc.vector.tensor_add(out[:], t1[:, :n], t2[:, :n])


def build_nc():
    nc = bacc.Bacc("TRN2", target_bir_lowering=False, debug=False,
                   num_devices=NC_)

    din = {}
    def dram_in(name, shape, dtype=BF16):
        din[name] = nc.dram_tensor(name, shape, dtype, kind="ExternalInput")
        return din[name]

    h0_d = dram_in("h0", [S, H])
    cos1_d = dram_in("cos1", [S, 128])
    sf1_d = dram_in("sf1", [S, 128])
    ident_d = dram_in("ident", [128, 128])
    cmask_d = dram_in("cmask", [128, 128])
    ones_d = dram_in("ones", [128, 1])
    for l in range(L):
        dram_in(f"qkvw{l}", [128, 32, 768], FP8)
        dram_in(f"ow{l}", [128, 4, H], FP8)
        dram_in(f"gw{l}", [3, 8, 128, 4, 512], FP8)   # [nb, kp, p, j, n]
        dram_in(f"uw{l}", [3, 8, 128, 4, 512], FP8)
        dram_in(f"dw{l}", [8, 3, 128, 4, 512], FP8)   # [n, tp, p, j, n]
    lmw_d = dram_in("lmw", [8, 8, 128, 4, 500], FP8)  # [vb, kp, p, j, n]
    wsel_d = dram_in("wsel", [H, S])

    gsum_o = nc.dram_tensor("gsum_o", [128, 8], F32, kind="ExternalOutput")
    tlog_o = nc.dram_tensor("tlog_o", [1, S], F32, kind="ExternalOutput")

    rg = [list(range(NC_))]

    with tile.TileContext(nc) as tc:
        with (
            tc.tile_pool(name="pconst", bufs=1) as pconst,
            tc.tile_pool(name="psmall", bufs=1) as psmall,
            tc.tile_pool(name="pbridge", bufs=1) as pbridge,
            tc.tile_pool(name="pdram", bufs=1, space="DRAM") as pdram,
        ):
            ident_sb = pconst.tile([128, 128], BF16)
            cmask_sb = pconst.tile([128, 128], BF16)
            ones_sb = pconst.tile([128, 1], BF16)
            cos_sb = pconst.tile([128, 8, 128], BF16)
            sf_sb = pconst.tile([128, 8, 128], BF16)
            negSM = pconst.tile([128, 1], F32)
            nc.any.memset(negSM[:], -SC_MAX)
            nc.sync.dma_start(ident_sb[:], ident_d.ap())
            nc.sync.dma_start(cmask_sb[:], cmask_d.ap())
            nc.sync.dma_start(ones_sb[:], ones_d.ap())
            for i in range(8):
                nc.sync.dma_start(cos_sb[:, i, :], cos1_d.ap()[i * 128:(i + 1) * 128, :])
                nc.sync.dma_start(sf_sb[:, i, :], sf1_d.ap()[i * 128:(i + 1) * 128, :])

            xnt = [pbridge.tile([128, 32, 128], FP8, name=f"xnt_{j}")
                   for j in range(8)]

            hstack = ExitStack()
            phh = hstack.enter_context(tc.tile_pool(name="phh", bufs=1))
            h_sb = phh.tile([128, 8, H], BF16)
            for i in range(8):
                nc.sync.dma_start(h_sb[:, i, :], h0_d.ap()[i * 128:(i + 1) * 128, :])

            # quarter-grained AR buffers: [4 quarters][256, H]
            ar_ins, ar_outss, ar2_ins, ar2_outss = [], [], [], []
            for l in range(L):
                ar_ins.append(pdram.tile([S, H], BF16, name=f"ar_in_{l}"))
                ar_outss.append([pdram.tile([256, H], BF16, addr_space="Shared",
                                            name=f"ar_out_{l}_{q}")
                                 for q in range(4)])
                ar2_ins.append(pdram.tile([S, H], BF16, name=f"ar2_in_{l}"))
                ar2_outss.append([pdram.tile([256, H], BF16, addr_space="Shared",
                                             name=f"ar2_out_{l}_{q}")
                                  for q in range(4)])

            def prep(pool, psum, j, res_q, dst, uid, nt_tag="mix", nt_bufs=2):
                """h_sb[:,j] += AR-quarter residual; dst = norm-transpose."""
                if res_q is not None:
                    rt = pool.tile([128, H], BF16, tag="prep_rt", bufs=2,
                                   name=f"rt_{uid}")
                    nc.sync.dma_start(
                        rt[:], res_q[(j % 2) * 128:(j % 2 + 1) * 128, :])
                    nc.vector.tensor_add(h_sb[:, j, :], h_sb[:, j, :], rt[:])
                _norm_transpose(nc, psmall, pool, psum, h_sb[:, j, :], dst,
                                ident_sb, uid, nt_tag=nt_tag, nt_bufs=nt_bufs)

            xfstack = ExitStack()

            for l in range(L):
                # ======== attention: per-tile qkv -> heads -> o-proj ========
                with (
                    tc.tile_pool(name="pal", bufs=1) as pal,
                    tc.tile_pool(name="paps", bufs=1, space="PSUM") as paps,
                ):
                    kT_sb = pal.tile([128, S], BF16)
                    v_sb = pal.tile([128, 8, 128], BF16)
                    ar_in = ar_ins[l]
                    ar_outs = ar_outss[l]
                    wqkv_sb = pal.tile([128, 32, 768], FP8)
                    ow_sb = pal.tile([128, 4, H], FP8)
                    nc.sync.dma_start(wqkv_sb[:], din[f"qkvw{l}"].ap())
                    nc.sync.dma_start(ow_sb[:], din[f"ow{l}"].ap())
                    if l == 0:
                        for j in range(8):
                            prep(pal, paps, j, None, xnt[j], f"i{j}")
                    for i in range(8):
                        if l > 0 and i in (0, 1, 3, 4):
                            # prep the second-half attention tiles of this
                            # layer as their ar2 quarters land (q3 is late)
                            j = {0: 4, 1: 5, 3: 6, 4: 7}[i]
                            prep(pal, paps, j, ar2_outss[l - 1][j // 2],
                                 xnt[j], f"a{l}_{j}")
                        psq = paps.tile([128, 512], F32, tag="psq", bufs=2,
                                        name=f"psq_{l}_{i}")
                        pskv = paps.tile([128, 256], F32, tag="pskv", bufs=1,
                                         name=f"pskv_{l}_{i}")
                        for k in range(16):
                            nc.tensor.matmul(psq[:], xnt[i][:, 2 * k:2 * k + 2, :],
                                             wqkv_sb[:, 2 * k:2 * k + 2, 0:512],
                                             start=(k == 0), stop=(k == 15),
                                             perf_mode=DR)
                            nc.tensor.matmul(pskv[:], xnt[i][:, 2 * k:2 * k + 2, :],
                                             wqkv_sb[:, 2 * k:2 * k + 2, 512:768],
                                             start=(k == 0), stop=(k == 15),
                                             perf_mode=DR)
                        qT_sb = pal.tile([128, 4, 128], BF16, tag="qT",
                                         bufs=2, name=f"qT_{l}_{i}")
                        oT_sb = pal.tile([128, 4, 128], FP8, tag="oT",
                                         bufs=2, name=f"oT_{l}_{i}")
                        q_rot = pal.tile([128, 512], BF16, tag="q_rot", bufs=2,
                                         name=f"qr_{l}_{i}")
                        k_rot = pal.tile([128, 128], BF16, tag="k_rot", bufs=2,
                                         name=f"kr_{l}_{i}")
                        _rope(nc, pal, psq[:], cos_sb[:, i, :], sf_sb[:, i, :],
                              q_rot, 4, f"{l}_{i}")
                        _rope(nc, pal, pskv[:, 0:128], cos_sb[:, i, :],
                              sf_sb[:, i, :], k_rot, 1, f"{l}_{i}")
                        nc.vector.tensor_scalar_mul(v_sb[:, i, :],
                                                    pskv[:, 128:256], 1.0 / WS)
                        ptr = paps.tile([128, 512], F32, tag="mix", bufs=2,
                                        name=f"ptrq_{l}_{i}")
                        for hh in range(4):
                            nc.tensor.matmul(ptr[:, hh * 128:(hh + 1) * 128],
                                             q_rot[:, hh * 128:(hh + 1) * 128],
                                             ident_sb[:], start=True, stop=True)
                        nc.any.tensor_copy(qT_sb[:],
                                           ptr[:].rearrange("p (j m) -> p j m", j=4))
                        ptrk = paps.tile([128, 512], F32, tag="mix", bufs=2,
                                         name=f"ptrk_{l}_{i}")
                        nc.tensor.matmul(ptrk[:, :128], k_rot[:], ident_sb[:],
                                         start=True, stop=True)
                        nc.any.tensor_copy(kT_sb[:, i * 128:(i + 1) * 128], ptrk[:, :128])
                        n2 = 128 * (i + 1)
                        for hh in range(4):
                            pss = paps.tile([128, 1024], F32, tag="pss", bufs=1,
                                            name=f"pss_{l}_{hh}_{i}")
                            lhs_q = qT_sb[:, hh, :]
                            c0 = 0
                            while c0 < n2 - 128:
                                N = min(512, n2 - 128 - c0)
                                nc.tensor.matmul(pss[:, c0:c0 + N], lhs_q,
                                                 kT_sb[:, c0:c0 + N],
                                                 start=True, stop=True)
                                c0 += N
                            nc.tensor.matmul(pss[:, n2 - 128:n2], lhs_q,
                                             kT_sb[:, n2 - 128:n2],
                                             start=True, stop=False)
                            nc.tensor.matmul(pss[:, n2 - 128:n2], ident_sb[:],
                                             cmask_sb[:], start=False, stop=True)
                            sume = psmall.tile([128, 1], F32, tag="sume", bufs=2,
                                               name=f"sume_{l}_{hh}_{i}")
                            exp_sb = pal.tile([128, 1024], BF16, tag="exp", bufs=2,
                                              name=f"exp_{l}_{hh}_{i}")
                            nc.scalar.activation(exp_sb[:, :n2], pss[:, :n2], AF.Exp,
                                                 bias=negSM[:], accum_out=sume[:])
                            rec = psmall.tile([128, 1], F32, tag="rec", bufs=2,
                                              name=f"rec_{l}_{hh}_{i}")
                            nc.vector.reciprocal(rec[:], sume[:])
                            diag_r = pal.tile([128, 128], BF16, tag="diag_r", bufs=2,
                                              name=f"diagr_{l}_{hh}_{i}")
                            nc.vector.tensor_scalar_mul(diag_r[:], ident_sb[:], rec[:])
                            atcol = pal.tile([128, 8, 128], BF16, tag="atcol", bufs=2,
                                             name=f"atcol_{l}_{hh}_{i}")
                            for jg in range(0, i + 1, 4):
                                ng = min(4, i + 1 - jg)
                                pat = paps.tile([128, 512], F32, tag="mix", bufs=2,
                                                name=f"pat_{l}_{hh}_{i}_{jg}")
                                for jj in range(ng):
                                    j = jg + jj
                                    nc.tensor.matmul(pat[:, jj * 128:(jj + 1) * 128],
                                                     exp_sb[:, j * 128:(j + 1) * 128],
                                                     diag_r[:], start=True, stop=True)
                                nc.any.tensor_copy(
                                    atcol[:, jg:jg + ng, :],
                                    pat[:, :ng * 128].rearrange("p (j m) -> p j m", j=ng))
                            pso = paps.tile([128, 128], F32, tag="pso", bufs=1,
                                            name=f"pso_{l}_{hh}_{i}")
                            for j in range(i + 1):
                                nc.tensor.matmul(pso[:], v_sb[:, j, :], atcol[:, j, :],
                                                 start=(j == 0), stop=(j == i))
                            nc.any.tensor_copy(oT_sb[:, hh, :], pso[:])
                        ob = pal.tile([128, H], BF16, tag="ob", bufs=1,
                                      name=f"ob_{l}_{i}")
                        for n in range(8):
                            pps = paps.tile([128, 512], F32, tag="mix", bufs=2,
                                            name=f"pop_{l}_{i}_{n}")
                            for t in range(2):
                                nc.tensor.matmul(pps[:], oT_sb[:, 2 * t:2 * t + 2, :],
                                                 ow_sb[:, 2 * t:2 * t + 2,
                                                       n * 512:(n + 1) * 512],
                                                 start=(t == 0), stop=(t == 1),
                                                 perf_mode=DR)
                            nc.vector.tensor_scalar_mul(ob[:, n * 512:(n + 1) * 512],
                                                        pps[:], 1.0 / WS)
                        nc.sync.dma_start(ar_in[i * 128:(i + 1) * 128, :], ob[:])
                        if i % 2 == 1:
                            q = i // 2
                            nc.gpsimd.collective_compute(
                                "AllReduce", ALU.add, replica_groups=rg,
                                ins=[ar_in[q * 256:(q + 1) * 256, :].opt()],
                                outs=[ar_outs[q].opt()])
                        if i >= 4:
                            # prep MLP tiles 0..3 (attn AR quarters 0,1 ready)
                            j = i - 4
                            prep(pal, paps, j, ar_outs[j // 2], xnt[j],
                                 f"m{l}_{j}")
                    # prep MLP tiles 4,5 (quarter 2 posted after i==5)
                    for j in (4, 5):
                        prep(pal, paps, j, ar_outs[j // 2], xnt[j], f"m{l}_{j}")

                # ===== MLP: gate/up -> down in row-quarters -> AR2 ==========
                if l == L - 1:
                    pxf = xfstack.enter_context(tc.tile_pool(name="pxf", bufs=1))
                    xf_sb = pxf.tile([128, 32, S], FP8)
                with (
                    tc.tile_pool(name="pml", bufs=1) as pml,
                    tc.tile_pool(name="pmps", bufs=1, space="PSUM") as pmps,
                ):
                    ar2_in = ar2_ins[l]
                    ar2_outs = ar2_outss[l]

                    def prep_next(j):
                        """prep for the next phase: attention l+1 (into xnt)
                        or the final norm (into xf)."""
                        if l < L - 1:
                            prep(pml, pmps, j, ar2_outs[j // 2], xnt[j],
                                 f"a{l + 1}_{j}", nt_tag="mlpps", nt_bufs=4)
                        else:
                            prep(pml, pmps, j, ar2_outs[j // 2],
                                 xf_sb[:, :, j * 128:(j + 1) * 128],
                                 f"f{j}", nt_tag="mlpps", nt_bufs=4)

                    for ig in range(2):
                        with tc.tile_pool(name="pgu", bufs=1) as pgu:
                            if ig == 1:
                                # prep MLP tiles 6,7 (attn AR q3 long done)
                                for j in (6, 7):
                                    prep(pml, pmps, j, ar_outs[j // 2], xnt[j],
                                         f"m{l}_{j}", nt_tag="mlpps", nt_bufs=4)
                            yt_sb = pml.tile([128, 12, 512], FP8, tag="yt",
                                             bufs=2, name=f"yt_{l}_{ig}")
                            nc.vector.memset(yt_sb[:, 11, :], 0.0)
                            gu = {}
                            for wi, (wname, tag) in enumerate(
                                    ((f"gw{l}", "g"), (f"uw{l}", "u"))):
                                outs = [pgu.tile([128, IP], BF16, tag=tag, bufs=4,
                                                 name=f"{tag}_{l}_{ig}_{ii}")
                                        for ii in range(4)]
                                gu[tag] = outs
                                for nb in range(3):
                                    NB = 512 if nb < 2 else IP - 1024
                                    pg = [pmps.tile([128, 512], F32, tag="mlpps", bufs=4,
                                                    name=f"pg_{l}_{ig}_{tag}_{nb}_{ii}")
                                          for ii in range(4)]
                                    for kp in range(8):
                                        wt = pgu.tile([128, 4, 512], FP8, tag="wstream",
                                                      bufs=4,
                                                      name=f"wt_{l}_{ig}_{tag}_{nb}_{kp}")
                                        nc.sync.dma_start(wt[:], din[wname].ap()[nb, kp])
                                        for jp in range(2):
                                            k = kp * 4 + 2 * jp
                                            for ii in range(4):
                                                nc.tensor.matmul(
                                                    pg[ii][:, :NB],
                                                    xnt[ig * 4 + ii][:, k:k + 2, :],
                                                    wt[:, 2 * jp:2 * jp + 2, :NB],
                                                    start=(k == 0), stop=(k == 30),
                                                    perf_mode=DR)
                                    for ii in range(4):
                                        nc.any.tensor_copy(
                                            outs[ii][:, nb * 512:nb * 512 + NB],
                                            pg[ii][:, :NB])
                                    # interleave next-phase preps into ig=1
                                    if ig == 1 and wi == 0:
                                        prep_next(nb)          # tiles 0,1,2
                                    if ig == 1 and wi == 1 and nb == 0:
                                        prep_next(3)
                            for iq in range(2):
                                for ii in (iq * 2, iq * 2 + 1):
                                    i = ig * 4 + ii
                                    ysil = pgu.tile([128, IP], BF16, tag="ysil", bufs=2,
                                                    name=f"ysil_{l}_{i}")
                                    nc.scalar.activation(ysil[:], gu["g"][ii][:], AF.Silu,
                                                         scale=1.0 / WS)
                                    y = gu["u"][ii]
                                    nc.vector.tensor_mul(y[:], ysil[:], y[:])
                                    for tq in range(3):
                                        ts = [tq * 4 + j for j in range(4) if tq * 4 + j < 11]
                                        ptr = pmps.tile([128, 512], F32, tag="mlpps", bufs=4,
                                                        name=f"ytr_{l}_{i}_{tq}")
                                        for jj, t in enumerate(ts):
                                            nc.tensor.matmul(ptr[:, jj * 128:(jj + 1) * 128],
                                                             y[:, t * 128:(t + 1) * 128],
                                                             ident_sb[:], start=True, stop=True)
                                        nc.any.tensor_copy(
                                            yt_sb[:, ts[0]:ts[0] + len(ts),
                                                  ii * 128:(ii + 1) * 128],
                                            ptr[:, :len(ts) * 128].rearrange(
                                                "p (j m) -> p j m", j=len(ts)))
                                for n in range(8):
                                    pd = [pmps.tile([128, 512], F32, tag=f"pd{i2}", bufs=1,
                                                    name=f"pd_{l}_{ig}_{iq}_{n}_{i2}")
                                          for i2 in range(2)]
                                    for tp in range(3):
                                        dwt = pgu.tile([128, 4, 512], FP8, tag="dwstream",
                                                       bufs=4,
                                                       name=f"dwt_{l}_{ig}_{iq}_{n}_{tp}")
                                        nc.sync.dma_start(dwt[:], din[f"dw{l}"].ap()[n, tp])
                                        for jp in range(2):
                                            c = tp * 4 + 2 * jp
                                            for i2 in range(2):
                                                ii = iq * 2 + i2
                                                nc.tensor.matmul(
                                                    pd[i2][:],
                                                    yt_sb[:, c:c + 2,
                                                          ii * 128:(ii + 1) * 128],
                                                    dwt[:, 2 * jp:2 * jp + 2, :],
                                                    start=(c == 0), stop=(c == 10),
                                                    perf_mode=DR)
                                    for i2 in range(2):
                                        i = ig * 4 + iq * 2 + i2
                                        db = pgu.tile([128, 512], BF16, tag="db", bufs=2,
                                                      name=f"db_{l}_{ig}_{iq}_{n}_{i2}")
                                        nc.vector.tensor_scalar_mul(db[:], pd[i2][:],
                                                                    1.0 / (WS * US))
                                        nc.sync.dma_start(
                                            ar2_in[i * 128:(i + 1) * 128,
                                                   n * 512:(n + 1) * 512], db[:])
                                q = ig * 2 + iq
                                nc.gpsimd.collective_compute(
                                    "AllReduce", ALU.add, replica_groups=rg,
                                    ins=[ar2_in[q * 256:(q + 1) * 256, :].opt()],
                                    outs=[ar2_outs[q].opt()])
                    if l == L - 1:
                        # final xf tiles 4..7 (ar2 q2/q3 land late) with the
                        # tlog halves interleaved to hide the tail AR latency
                        with (
                            tc.tile_pool(name="ptl", bufs=1) as ptl,
                            tc.tile_pool(name="ptps", bufs=1, space="PSUM") as ptps,
                        ):
                            pts = [ptps.tile([1, 512], F32, name=f"pt{h_}")
                                   for h_ in range(2)]

                            def tlog_half(half):
                                for kp in range(8):
                                    ws = ptl.tile([128, 4, 512], BF16, tag="wsel",
                                                  bufs=2, name=f"ws_{half}_{kp}")
                                    nc.sync.dma_start(
                                        ws[:], wsel_d.ap()[kp * 512:(kp + 1) * 512,
                                                           half * 512:(half + 1) * 512]
                                        .rearrange("(j p) n -> p j n", p=128))
                                    for jk in range(4):
                                        k = kp * 4 + jk
                                        tm = ptl.tile([128, 512], BF16, tag="tm",
                                                      bufs=2, name=f"tm_{half}_{k}")
                                        nc.vector.tensor_mul(
                                            tm[:], xf_sb[:, k,
                                                         half * 512:(half + 1) * 512],
                                            ws[:, jk, :])
                                        nc.tensor.matmul(pts[half][:], ones_sb[:],
                                                         tm[:], start=(k == 0),
                                                         stop=(k == 31))

                            prep_next(4)
                            prep_next(5)
                            tlog_half(0)
                            prep_next(6)
                            prep_next(7)
                            tlog_half(1)
                            tl_sb = ptl.tile([1, S], F32)
                            nc.any.tensor_copy(tl_sb[:, :512], pts[0][:])
                            nc.any.tensor_copy(tl_sb[:, 512:], pts[1][:])
                            nc.sync.dma_start(tlog_o.ap(), tl_sb[:])

            # ==================== lm (online fixed-max softmax) =============

            with (
                tc.tile_pool(name="plm", bufs=1) as plm,
                tc.tile_pool(name="plps", bufs=1, space="PSUM") as plps,
                tc.tile_pool(name="pld", bufs=1, space="DRAM") as pld,
            ):
                s_sb = plm.tile([128, 8], F32)
                nc.any.memset(s_sb[:], 0.0)
                negM = plm.tile([128, 1], F32)
                nc.any.memset(negM[:], -LM_MAX)
                for vb in range(8):
                    pl = [plps.tile([128, 500], F32, tag=f"pl{i}", bufs=1,
                                    name=f"pl_{vb}_{i}") for i in range(8)]
                    for kp in range(8):
                        lt = plm.tile([128, 4, 500], FP8, tag="lmw", bufs=4,
                                      name=f"lt_{vb}_{kp}")
                        nc.sync.dma_start(lt[:], lmw_d.ap()[vb, kp])
                        for jp in range(2):
                            k = kp * 4 + 2 * jp
                            for i in range(8):
                                nc.tensor.matmul(pl[i][:],
                                                 xf_sb[:, k:k + 2, i * 128:(i + 1) * 128],
                                                 lt[:, 2 * jp:2 * jp + 2, :],
                                                 start=(k == 0), stop=(k == 30),
                                                 perf_mode=DR)
                    for i in range(8):
                        se = psmall.tile([128, 1], F32, tag="se", bufs=2,
                                         name=f"se_{vb}_{i}")
                        scr = plm.tile([128, 500], BF16, tag="scr", bufs=2,
                                       name=f"scr_{vb}_{i}")
                        nc.scalar.activation(scr[:], pl[i][:], AF.Exp,
                                             bias=negM[:], scale=1.0 / WS,
                                             accum_out=se[:])
                        nc.vector.tensor_add(s_sb[:, i:i + 1], s_sb[:, i:i + 1],
                                             se[:])
                gs_in = pld.tile([128, 8], F32)
                gs_out = pld.tile([128, 8], F32, addr_space="Shared")
                nc.sync.dma_start(gs_in[:], s_sb[:])
                nc.gpsimd.collective_compute("AllReduce", ALU.add, replica_groups=rg,
                                             ins=[gs_in.opt()], outs=[gs_out.opt()])
                gsf_sb = plm.tile([128, 8], F32)
                nc.sync.dma_start(gsf_sb[:], gs_out[:])
                nc.sync.dma_start(gsum_o.ap(), gsf_sb[:])
            xfstack.close()
            hstack.close()

    nc.compile()
    return nc


# ------------------------------------------------------------------- host --

def _to_f8(x):
    return np.clip(x, -240.0, 240.0).astype(f8)


def host_prep(inputs):
    inp = {k: np.asarray(v) for k, v in inputs.items()}
    embed = inp["embed"].astype(np.float32)
    ids = inp["input_ids"].reshape(-1).astype(np.int64)
    labels = inp["labels"].reshape(-1).astype(np.int64)

    h = embed[ids]
    cw = inp["conv_w"].astype(np.float32)
    logit = h[:-1] @ cw[0, :H] + h[1:] @ cw[0, H:] + np.float32(inp["conv_b"][0])
    mask = logit > 0
    m = np.concatenate([mask, [False]])
    hn = np.where(m[:, None], 0.5 * (h + np.roll(h, -1, axis=0)), h)
    keep = np.concatenate([[True], ~mask])
    order = np.argsort(~keep, kind="stable")
    h0 = hn[order]
    lab = labels[order]
    valid_len = int(keep.sum())

    inv = 1.0 / (THETA ** (np.arange(0, HD, 2, dtype=np.float32) / HD))
    t = np.arange(S, dtype=np.float32)
    freqs = np.outer(t, inv)
    emb = np.concatenate([freqs, freqs], -1)
    cos, sin = np.cos(emb), np.sin(emb)
    sinflip = np.concatenate([-sin[:, :HD // 2], sin[:, HD // 2:]], -1)
    # rope constants absorb the 1/WS compensation for the fp8 q/k weights
    cos1 = (cos / WS).astype(bf16)
    sf1 = (sinflip / WS).astype(bf16)

    ident = np.eye(128, dtype=bf16)
    cmask = np.where(np.arange(128)[None, :] > np.arange(128)[:, None],
                     np.float32(NEG), np.float32(0)).astype(bf16)
    ones = np.ones((128, 1), dtype=bf16)

    ln1 = inp["ln1_w"].astype(np.float32)
    ln2 = inp["ln2_w"].astype(np.float32)
    normw = inp["norm_w"].astype(np.float32)
    qsc = np.float32(1.0 / np.sqrt(HD))
    lm_folded = normw[:, None] * inp["lm_head_w"].astype(np.float32)
    lm_q = _to_f8(lm_folded * WS)          # quantized once, reused for wsel
    tgt = np.concatenate([lab[1:], [0]]).astype(np.int64)
    wsel = np.ascontiguousarray(lm_q.astype(np.float32)[:, tgt] / WS).astype(bf16)

    common = dict(h0=h0.astype(bf16), cos1=cos1, sf1=sf1, ident=ident,
                  cmask=cmask, ones=ones, wsel=wsel)
    in_maps = []
    for c in range(NC_):
        mcore = dict(common)
        for l in range(L):
            qw = ln1[l][:, None] * inp["q_w"][l].astype(np.float32) * qsc * WS
            kw = ln1[l][:, None] * inp["k_w"][l].astype(np.float32) * WS
            vw = ln1[l][:, None] * inp["v_w"][l].astype(np.float32) * WS
            gw = ln2[l][:, None] * inp["gate_w"][l].astype(np.float32) * WS
            uw = ln2[l][:, None] * inp["up_w"][l].astype(np.float32) * US
            dw = inp["down_w"][l].astype(np.float32) * WS
            qkv = np.concatenate(
                [qw[:, c * 512:(c + 1) * 512],
                 kw[:, c * 128:(c + 1) * 128],
                 vw[:, c * 128:(c + 1) * 128]], 1)          # [H, 768]
            mcore[f"qkvw{l}"] = np.ascontiguousarray(
                _to_f8(qkv).reshape(32, 128, 768).transpose(1, 0, 2))
            ow = inp["o_w"][l][c * 512:(c + 1) * 512].astype(np.float32) * WS
            mcore[f"ow{l}"] = np.ascontiguousarray(
                _to_f8(ow).reshape(4, 128, H).transpose(1, 0, 2))
            gws = np.zeros((H, IP), np.float32)
            uws = np.zeros((H, IP), np.float32)
            dws = np.zeros((IP2, H), np.float32)
            gws[:, :IPC] = gw[:, c * IPC:(c + 1) * IPC]
            uws[:, :IPC] = uw[:, c * IPC:(c + 1) * IPC]
            dws[:IPC] = dw[c * IPC:(c + 1) * IPC]
            for wname, warr in ((f"gw{l}", gws), (f"uw{l}", uws)):
                out = np.zeros((3, 8, 128, 4, 512), np.float32)
                for nb in range(3):
                    NBc = 512 if nb < 2 else IP - 1024
                    blk = warr[:, nb * 512:nb * 512 + NBc]       # [H, NBc]
                    out[nb, :, :, :, :NBc] = blk.reshape(
                        8, 4, 128, NBc).transpose(0, 2, 1, 3)
                mcore[wname] = _to_f8(out)
            dout = np.zeros((8, 3, 128, 4, 512), np.float32)
            for n in range(8):
                blk = dws[:, n * 512:(n + 1) * 512]              # [IP2, 512]
                dout[n] = blk.reshape(3, 4, 128, 512).transpose(0, 2, 1, 3)
            mcore[f"dw{l}"] = _to_f8(dout)
        lmc = lm_q[:, c * VS:(c + 1) * VS].astype(np.float32)    # [H, 4000]
        lout = np.zeros((8, 8, 128, 4, 500), np.float32)
        for vb in range(8):
            blk = lmc[:, vb * 500:(vb + 1) * 500]                # [H, 500]
            lout[vb] = blk.reshape(8, 4, 128, 500).transpose(0, 2, 1, 3)
        mcore["lmw"] = _to_f8(lout)
        in_maps.append(mcore)

    return in_maps, valid_len


def kernel(**inputs) -> np.ndarray:
    in_maps, valid_len = host_prep(inputs)
    if "nc" not in _cache:
        _cache["nc"] = build_nc()
    nc = _cache["nc"]
    res = run_bass_kernel_spmd(nc, in_maps, list(range(NC_)),
                               **last_run_info.get("run_kwargs", {}))
    last_run_info["res"] = res
    out = res.results[0]
    gsum = out["gsum_o"].transpose(1, 0).reshape(S).astype(np.float64)
    tlog = out["tlog_o"].reshape(S).astype(np.float64)
    ce = LM_MAX + np.log(gsum) - tlog
    w = (np.arange(S - 1) < valid_len - 1).astype(np.float64)
    loss = (ce[:S - 1] * w).sum() / w.sum()
    return np.float32(loss)
